# revision 43
# baseline (speedup 1.0000x reference)
"""Distributed Trainium2 kernel for nn_ADUPredictor (12-layer sliding-window
encoder + BiLSTM/attention-pool tail), SPMD across 8 NeuronCores.

Sharding: cores 0-3 = batch 0, cores 4-7 = batch 1; core g owns the 512-token
sequence quarter q = g%4 plus 256-token halos on each side. Activations are
kept feature-major ("T layout", [feature, token]) so every projection uses the
weight as the stationary matmul operand. Per-layer halo exchange = AllGather of
each core's own x (bf16) + indirect-DMA picks of the two neighbor boundary
blocks. The BiLSTM/pool tail runs per seq-quarter (both batches) with one small
AllReduce for the global softmax pooling; LSTM2 + tag head are replicated.
"""

import sys

if "/opt/trn_rl_repo" not in sys.path:
    sys.path.insert(0, "/opt/trn_rl_repo")

import numpy as np
import ml_dtypes

import concourse.bass as bass
import concourse.mybir as mybir
import concourse.tile as tile
from concourse import bacc
from concourse.bass import ds, ts
from concourse.bass_utils import run_bass_kernel_spmd
from concourse.masks import make_identity
from concourse.alu_op_type import AluOpType
import bass_rust

F32 = mybir.dt.float32
F8 = mybir.dt.float8e4
DR = mybir.MatmulPerfMode.DoubleRow
W2SCALE = 16.0
BF16 = mybir.dt.float16
I32 = mybir.dt.int32
AF = mybir.ActivationFunctionType
AX = bass_rust.AxisListType

B, S = 2, 2048
D, H, DH, W, LMAX, FF = 768, 12, 64, 256, 12, 3072
VOCAB = 50265
HID, HALF = 512, 256
KT = D // 128          # 6 feature tiles
RT = 8                 # 1024 rows / 128
OWN = 512              # own rows per core
R = 1024               # own + halos
NC = 8
EPS = 1e-5
MASKVAL = -50.0

BF = np.float16


def build_nc(L=LMAX):
    nc = bacc.Bacc(None)

    def param(name, shape, dt):
        return nc.declare_dram_parameter(name, list(shape), dt, isOutput=False)

    sent_idx = param("sent_idx", [R, 1], I32)
    pos_sl = param("pos_sl", [R, D], F32)
    masks = param("masks", [2 * 4 * 128, 256], BF16)
    picks = param("picks", [4 * KT * 128, 1], I32)

    word_emb = param("word_emb", [VOCAB, D], BF16)
    embs = param("embs", [1, D], F32)
    embb = param("embb", [1, D], F32)
    wq = param("wq", [L, D, D], BF16)
    wk = param("wk", [L, D, D], BF16)
    wv = param("wv", [L, D, D], BF16)
    wo = param("wo", [L, D, D], BF16)
    bq = param("bq", [L, D], F32)
    bk = param("bk", [L, D], F32)
    bo = param("bo", [L, D], F32)
    ln1s = param("ln1s", [L, D], F32)
    ln1b = param("ln1b", [L, D], F32)
    ln2s = param("ln2s", [L, D], F32)
    ln2b = param("ln2b", [L, D], F32)
    wf1 = param("wf1", [L, D, FF], BF16)
    bf1 = param("bf1", [L, FF], F32)
    wf2 = param("wf2", [L, FF, D], BF16)
    bf2 = param("bf2", [L, D], F32)

    l1wih = param("l1wih", [2, D, 4 * HALF], BF16)     # transposed [in, 4*HALF]
    l1whh = param("l1whh", [2, HALF, 4 * HALF], BF16)
    l1bih = param("l1bih", [2, 4 * HALF], F32)
    l1bhh = param("l1bhh", [2, 4 * HALF], F32)
    paw1 = param("paw1", [HID, 64], BF16)
    pab1 = param("pab1", [64, 1], F32)
    paw2 = param("paw2", [64, 1], BF16)
    pab2 = param("pab2", [1, 1], F32)
    l2wih = param("l2wih", [2, HID, 4 * HALF], BF16)
    l2whh = param("l2whh", [2, HALF, 4 * HALF], BF16)
    l2bih = param("l2bih", [2, 4 * HALF], F32)
    l2bhh = param("l2bhh", [2, 4 * HALF], F32)
    wtag = param("wtag", [HID, 1], BF16)
    btag = param("btag", [1, 1], F32)

    out = nc.declare_dram_parameter("out", [1, 2], F32, isOutput=True)

    with tile.TileContext(nc) as tc:
        with (
            tc.tile_pool(name="pers", bufs=1) as pers,
            tc.tile_pool(name="sb", bufs=1) as sb,
            tc.tile_pool(name="ps_sps", bufs=2, space="PSUM") as ps_sps,
            tc.tile_pool(name="ps_ctx", bufs=2, space="PSUM") as ps_ctx,
            tc.tile_pool(name="ps_big", bufs=2, space="PSUM") as ps_big,
            tc.tile_pool(name="dram", bufs=2, space="DRAM") as dram,
        ):
            # ---------------- persistent constants ----------------
            ones_col = pers.tile([128, 1], BF16)   # lhsT for partition sums
            nc.vector.memset(ones_col[:], 1.0)
            ones_row_f = pers.tile([1, 128], F32)  # lhsT for PE row-broadcasts
            nc.vector.memset(ones_row_f[:], 1.0)
            ones_row_bf = pers.tile([1, 128], BF16)
            nc.vector.memset(ones_row_bf[:], 1.0)
            ident = pers.tile([128, 128], BF16)
            make_identity(nc, ident[:])

            v_buf = pers.tile([128, RT, H * 65], BF16)
            nc.vector.memset(
                v_buf[:].rearrange("p r (h e) -> p (r h) e", e=65)[:, :, 64:65], 1.0
            )

            masks_sb = pers.tile([128, 8, 256], BF16)
            nc.sync.dma_start(
                out=masks_sb[:],
                in_=masks.ap().rearrange("(m p) q -> p m q", p=128),
            )

            idx_sb = pers.tile([128, RT, 1], I32)
            nc.sync.dma_start(
                out=idx_sb[:], in_=sent_idx.ap().rearrange("(t p) o -> p t o", p=128)
            )
            eps_sb = pers.tile([128, 1], F32)
            nc.vector.memset(eps_sb[:], EPS)
            embs_row = pers.tile([1, D], F32)
            nc.sync.dma_start(out=embs_row[:], in_=embs.ap())
            embb_row = pers.tile([1, D], F32)
            nc.sync.dma_start(out=embb_row[:], in_=embb.ap())
            embs_row_bf = pers.tile([1, D], BF16)
            nc.vector.tensor_copy(embs_row_bf[:], embs_row[:])
            embb_row_bf = pers.tile([1, D], BF16)
            nc.vector.tensor_copy(embb_row_bf[:], embb_row[:])

            # scratch tag helpers (sizes chosen as max over all users)
            def t_exp(name):   # 2KB slot: exp tiles, lstm gates, emb gathers
                return sb.tile([128, 512], F32, tag="exp", bufs=10, name=name)

            def t_exp_bf(name):
                return sb.tile([128, 512], BF16, tag="exp", bufs=10, name=name)

            # ---------------- embedding (per row-tile, streaming) ----------------
            x_bf = sb.tile([128, KT, R], BF16, tag="xbf", bufs=1, name="x_bf0")
            x_f = sb.tile([128, KT, OWN], BF16, tag="xf", bufs=1, name="x_f0")
            embs_full = sb.tile([128, D], BF16, tag="lnt", bufs=4, name="embs_full")
            nc.gpsimd.partition_broadcast(embs_full[:], embs_row_bf[:])
            embb_full = sb.tile([128, D], BF16, tag="lnt", bufs=4, name="embb_full")
            nc.gpsimd.partition_broadcast(embb_full[:], embb_row_bf[:])
            for rt in range(RT):
                xg = sb.tile([128, D], BF16, tag="exp", bufs=10, name="xg")
                nc.gpsimd.indirect_dma_start(
                    out=xg[:],
                    out_offset=None,
                    in_=word_emb.ap(),
                    in_offset=bass.IndirectOffsetOnAxis(ap=idx_sb[:, rt, :], axis=0),
                )
                pos_rt = sb.tile([128, D], F32, tag="wqkvo", bufs=2, name="pos_rt")
                nc.sync.dma_start(out=pos_rt[:], in_=pos_sl.ap()[ts(rt, 128), :])
                xe = sb.tile([128, D], F32, tag="wf1t", bufs=2, name="xe")
                nc.vector.tensor_tensor(
                    out=xe[:], in0=xg[:], in1=pos_rt[:], op=AluOpType.add
                )
                # row-wise LN over free axis (features)
                m = sb.tile([128, 1], F32, tag="lnrow", bufs=8, name="m")
                nc.vector.reduce_sum(m[:], xe[:], axis=AX.X)
                nc.scalar.mul(m[:], m[:], 1.0 / D)
                xc = sb.tile([128, D], F32, tag="wf1t", bufs=2, name="xc")
                nc.vector.tensor_scalar_sub(xc[:], xe[:], m[:, :1])
                sq = sb.tile([128, D], F32, tag="wqkvo", bufs=2, name="sq")
                nc.scalar.activation(sq[:], xc[:], AF.Square)
                v_ = sb.tile([128, 1], F32, tag="lnrow", bufs=8, name="v_")
                nc.vector.reduce_sum(v_[:], sq[:], axis=AX.X)
                lnv = sb.tile([128, 1], F32, tag="lnrow", bufs=8, name="lnv")
                nc.scalar.activation(lnv[:], v_[:], AF.Ln, bias=eps_sb[:, :1], scale=1.0 / D)
                rstd = sb.tile([128, 1], F32, tag="lnrow", bufs=8, name="rstd")
                nc.scalar.activation(rstd[:], lnv[:], AF.Exp, scale=-0.5)
                nc.vector.tensor_scalar_mul(xc[:], xc[:], rstd[:, :1])
                nc.vector.tensor_tensor(out=xc[:], in0=xc[:], in1=embs_full[:], op=AluOpType.mult)
                xn_bf = sb.tile([128, D], BF16, tag="exp", bufs=10, name="xn_bf")
                nc.vector.tensor_tensor(
                    out=xn_bf[:], in0=xc[:], in1=embb_full[:], op=AluOpType.add
                )
                for kt in range(KT):
                    tp = ps_sps.tile([128, 128], BF16, tag="sps", name="tp")
                    nc.tensor.transpose(tp[:], xn_bf[:, ts(kt, 128)], ident[:])
                    nc.scalar.activation(x_bf[:, kt, ts(rt, 128)], tp[:], AF.Copy)
                    if 2 <= rt <= 5:
                        nc.vector.tensor_copy(x_f[:, kt, ts(rt - 2, 128)], tp[:])

            # ---------------- helpers ----------------
            def load_wmat(p, l, name):
                t = sb.tile([128, KT, D], BF16, tag="wqkvo", bufs=2, name=name)
                nc.sync.dma_start(
                    out=t[:], in_=p.ap()[l].rearrange("(k p) m -> p k m", p=128)
                )
                return t

            def load_vec(p, l, name, scale=None):
                t = sb.tile([128, KT], F32, tag="pvec", bufs=14, name=name)
                nc.sync.dma_start(
                    out=t[:], in_=p.ap()[l].rearrange("(k p) -> p k", p=128)
                )
                if scale is not None:
                    nc.scalar.mul(t[:], t[:], scale)
                return t

            def ln_T(xr, s_sb, b_sb, out_f, out_bf, out_bf_off):
                """LayerNorm over the partition (feature) axis of xr [128,KT,OWN].

                If out_f is not None writes f32 result there and a bf16 copy to
                out_bf; else writes bf16 result directly to out_bf."""
                xr_bf = sb.tile([128, KT, OWN], BF16, tag="xrbf", bufs=1, name="xr_bf")
                nc.vector.tensor_copy(xr_bf[:], xr[:])
                xsq_bf = sb.tile([128, KT, OWN], BF16, tag="xsq", bufs=1, name="xsq_bf")
                nc.scalar.activation(xsq_bf[:], xr[:], AF.Square)
                sum_ps = ps_sps.tile([1, OWN], F32, tag="sps", name="sum_ps")
                sq_ps = ps_sps.tile([1, OWN], F32, tag="sps", name="sq_ps")
                for kt in range(KT):
                    nc.tensor.matmul(
                        sum_ps[:], lhsT=ones_col[:], rhs=xr_bf[:, kt, :],
                        start=(kt == 0), stop=(kt == KT - 1),
                    )
                for kt in range(KT):
                    nc.tensor.matmul(
                        sq_ps[:], lhsT=ones_col[:], rhs=xsq_bf[:, kt, :],
                        start=(kt == 0), stop=(kt == KT - 1),
                    )
                m = sb.tile([1, OWN], F32, tag="lncol", bufs=6, name="m")
                nc.scalar.activation(m[:], sum_ps[:], AF.Copy, scale=1.0 / D)
                msq = sb.tile([1, OWN], F32, tag="lncol", bufs=6, name="msq")
                nc.vector.tensor_tensor(out=msq[:], in0=m[:], in1=m[:], op=AluOpType.mult)
                var = sb.tile([1, OWN], F32, tag="lncol", bufs=6, name="var")
                nc.vector.scalar_tensor_tensor(
                    out=var[:], in0=sq_ps[:], scalar=1.0 / D, in1=msq[:],
                    op0=AluOpType.mult, op1=AluOpType.subtract,
                )
                lnv = sb.tile([1, OWN], F32, tag="lncol", bufs=6, name="lnv")
                nc.scalar.activation(lnv[:], var[:], AF.Ln, bias=eps_sb[:1, :1])
                # bf16 rows -> PE rank-1 broadcast -> bf16 SBUF copies; the whole
                # apply then runs at DVE 2x/4x bf16 rates
                rstd_rb = sb.tile([1, OWN], BF16, tag="lncol", bufs=6, name="rstd_rb")
                nc.scalar.activation(rstd_rb[:], lnv[:], AF.Exp, scale=-0.5)
                mrs_rb = sb.tile([1, OWN], BF16, tag="lncol", bufs=6, name="mrs_rb")
                nc.vector.tensor_tensor(out=mrs_rb[:], in0=m[:], in1=rstd_rb[:], op=AluOpType.mult)
                rstd_ps = ps_ctx.tile([128, OWN], F32, tag="ctx", name="rstd_ps")
                nc.tensor.matmul(
                    rstd_ps[:], lhsT=ones_row_bf[:1, :], rhs=rstd_rb[:1, :],
                    start=True, stop=True,
                )
                mrs_ps = ps_ctx.tile([128, OWN], F32, tag="ctx", name="mrs_ps")
                nc.tensor.matmul(
                    mrs_ps[:], lhsT=ones_row_bf[:1, :], rhs=mrs_rb[:1, :],
                    start=True, stop=True,
                )
                rstd_fb = sb.tile([128, OWN], BF16, tag="lnt", bufs=4, name="rstd_fb")
                nc.scalar.activation(rstd_fb[:], rstd_ps[:], AF.Copy)
                mrs_fb = sb.tile([128, OWN], BF16, tag="lnt", bufs=4, name="mrs_fb")
                nc.scalar.activation(mrs_fb[:], mrs_ps[:], AF.Copy)
                for kt in range(KT):
                    t1 = sb.tile([128, OWN], BF16, tag="lnap", bufs=3, name="t1")
                    nc.vector.tensor_tensor(
                        out=t1[:], in0=xr_bf[:, kt, :], in1=rstd_fb[:], op=AluOpType.mult
                    )
                    nc.vector.tensor_tensor(
                        out=t1[:], in0=t1[:], in1=mrs_fb[:], op=AluOpType.subtract
                    )
                    nc.vector.tensor_scalar(
                        out=out_bf[:, kt, ds(out_bf_off, OWN)], in0=t1[:],
                        scalar1=s_sb[:, kt : kt + 1], scalar2=b_sb[:, kt : kt + 1],
                        op0=AluOpType.mult, op1=AluOpType.add,
                    )
                    if out_f is not None:
                        nc.vector.tensor_copy(
                            out_f[:, kt, :], out_bf[:, kt, ds(out_bf_off, OWN)]
                        )

            # masked rt map: rt -> (masked chunk, mask slot)
            mask_slot = {0: (0, 0), 1: (0, 1), 2: (1, 0), 3: (1, 1),
                         4: (0, 2), 5: (0, 3), 6: (1, 2), 7: (1, 3)}

            # ---- per-core halo source rows, computed in SP registers:
            # q = pid%4; gl = pid-1+((4-q)//4); gr = pid+1-((q+1)//4); offsets *D
            pid = nc.sync.partition_id()
            q_r = nc.sync.alloc_register("q_r")
            nc.sync.reg_mod(q_r, pid, 4)
            t0_r = nc.sync.alloc_register("t0_r")
            nc.sync.reg_sub(t0_r, 4, q_r)
            nc.sync.reg_div(t0_r, t0_r, 4)          # 1 iff q==0
            gl_r = nc.sync.alloc_register("gl_r")
            nc.sync.reg_add(gl_r, pid, t0_r)
            nc.sync.reg_sub(gl_r, gl_r, 1)
            nc.sync.reg_mul(gl_r, gl_r, D)
            t1_r = nc.sync.alloc_register("t1_r")
            nc.sync.reg_add(t1_r, q_r, 1)
            nc.sync.reg_div(t1_r, t1_r, 4)          # 1 iff q==3
            gr_r = nc.sync.alloc_register("gr_r")
            nc.sync.reg_sub(gr_r, pid, t1_r)
            nc.sync.reg_add(gr_r, gr_r, 1)
            nc.sync.reg_mul(gr_r, gr_r, D)
            glD_sv = nc.sync.snap(gl_r, min_val=0, max_val=(NC - 1) * D)
            grD_sv = nc.sync.snap(gr_r, min_val=0, max_val=(NC - 1) * D)
            # tail gathers: rows q*D (batch 0) and (q+4)*D (batch 1)
            qD_r = nc.sync.alloc_register("qD_r")
            nc.sync.reg_mul(qD_r, q_r, D)
            q4D_r = nc.sync.alloc_register("q4D_r")
            nc.sync.reg_add(q4D_r, qD_r, 4 * D)
            qD_sv = nc.sync.snap(qD_r, min_val=0, max_val=3 * D)
            q4D_sv = nc.sync.snap(q4D_r, min_val=4 * D, max_val=7 * D)

            x_bf_cur, x_f_cur = x_bf, x_f

            for l in range(L):
                wq_sb = load_wmat(wq, l, "wq_sb")
                wk_sb = load_wmat(wk, l, "wk_sb")
                bq8 = load_vec(bq, l, "bq8", scale=0.125)
                bk_sb = load_vec(bk, l, "bk_sb")
                bo_sb = load_vec(bo, l, "bo_sb")
                ln1s_sb = load_vec(ln1s, l, "ln1s_sb")
                ln1b_sb = load_vec(ln1b, l, "ln1b_sb")
                ln2s_sb = load_vec(ln2s, l, "ln2s_sb")
                ln2b_sb = load_vec(ln2b, l, "ln2b_sb")
                bf2_sb = load_vec(bf2, l, "bf2_sb")
                bf1_sb = sb.tile([128, FF // 128], F32, tag="bf1", bufs=2, name="bf1_sb")
                nc.sync.dma_start(
                    out=bf1_sb[:], in_=bf1.ap()[l].rearrange("(k p) -> p k", p=128)
                )

                # ---- halo exchange for this layer's x (l==0 computed locally)
                if l > 0:
                  with tc.high_priority():
                      send = dram.tile([D, OWN], BF16, tag="send", name="send")
                      nc.sync.dma_start(
                          out=send.rearrange("(k p) c -> p k c", p=128),
                          in_=x_bf_cur[:, :, 256 : 256 + OWN],
                      )
                      gathered = dram.tile(
                          [NC * D, OWN], BF16, addr_space="Shared", tag="gath", name="gathered"
                      )
                      nc.gpsimd.collective_compute(
                          "AllGather",
                          AluOpType.bypass,
                          ins=[send.opt()],
                          outs=[gathered.opt()],
                          replica_groups=[list(range(NC))],
                      )
                      nc.sync.dma_start(
                          out=x_bf_cur[:, :, 0:256],
                          in_=gathered[ds(glD_sv, D), ds(256, 256)].rearrange(
                              "(k p) c -> p k c", p=128
                          ),
                      )
                      nc.sync.dma_start(
                          out=x_bf_cur[:, :, 768:1024],
                          in_=gathered[ds(grD_sv, D), ds(0, 256)].rearrange(
                              "(k p) c -> p k c", p=128
                          ),
                      )

                # ---- qT (own rows), kT own
                qT_sb = sb.tile([128, KT, OWN], BF16, tag="qT", bufs=1, name="qT_sb")
                kT_sb = sb.tile([128, KT, R], BF16, tag="kT", bufs=1, name="kT_sb")
                for mt in range(KT):
                    qp = ps_big.tile([128, OWN], F32, tag="big", name="qp")
                    for kt in range(KT):
                        nc.tensor.matmul(
                            qp[:], lhsT=wq_sb[:, kt, ts(mt, 128)],
                            rhs=x_bf_cur[:, kt, 256 : 256 + OWN],
                            start=(kt == 0), stop=(kt == KT - 1),
                        )
                    nc.vector.tensor_scalar(
                        out=qT_sb[:, mt, :], in0=qp[:],
                        scalar1=bq8[:, mt : mt + 1], scalar2=0.125,
                        op0=AluOpType.add, op1=AluOpType.mult,
                    )
                    kp = ps_big.tile([128, OWN], F32, tag="big", name="kp")
                    for kt in range(KT):
                        nc.tensor.matmul(
                            kp[:], lhsT=wk_sb[:, kt, ts(mt, 128)],
                            rhs=x_bf_cur[:, kt, 256 : 256 + OWN],
                            start=(kt == 0), stop=(kt == KT - 1),
                        )
                    nc.vector.tensor_scalar_add(
                        kT_sb[:, mt, 256 : 256 + OWN], kp[:], bk_sb[:, mt : mt + 1]
                    )

                wv_sb = load_wmat(wv, l, "wv_sb")

                def v_rows(rt):
                    for nh in range(2):
                        vp = ps_big.tile([128, 384], F32, tag="big", name="vp")
                        for kt in range(KT):
                            nc.tensor.matmul(
                                vp[:], lhsT=x_bf_cur[:, kt, ts(rt, 128)],
                                rhs=wv_sb[:, kt, ts(nh, 384)],
                                start=(kt == 0), stop=(kt == KT - 1),
                            )
                        nc.vector.tensor_copy(
                            v_buf[:, rt, :].rearrange("p (h e) -> p h e", e=65)[
                                :, ds(nh * 6, 6), 0:64
                            ],
                            vp[:].rearrange("p (h e) -> p h e", e=64),
                        )

                for rt in (2, 3, 4, 5):
                    v_rows(rt)

                # ---- halo kT, v (depend on picks)
                for mt in range(KT):
                    for side in range(2):
                        hoff = 0 if side == 0 else 768
                        kp2 = ps_big.tile([128, 256], F32, tag="big", name="kp2")
                        for kt in range(KT):
                            nc.tensor.matmul(
                                kp2[:], lhsT=wk_sb[:, kt, ts(mt, 128)],
                                rhs=x_bf_cur[:, kt, ds(hoff, 256)],
                                start=(kt == 0), stop=(kt == KT - 1),
                            )
                        nc.vector.tensor_scalar_add(
                            kT_sb[:, mt, ds(hoff if side == 0 else 768, 256)], kp2[:],
                            bk_sb[:, mt : mt + 1],
                        )
                for rt in (0, 1, 6, 7):
                    v_rows(rt)

                # ---- attention per head: scores+exp (8 rts) then ctx (2 chunks)
                ctxT_sb = sb.tile([128, KT, OWN], BF16, tag="ctxT", bufs=1, name="ctxT_sb")
                # unnormalized ctx stash (2 heads packed per partition group, slots
                # reused per 6-head half) + packed softmax denominators; reciprocal
                # runs batched over 12 (head, chunk) rows at a time
                ctxu_all = sb.tile([128, 3, 2, 256], F32, tag="ctxu", bufs=1, name="ctxu_all")
                # halves live at partition bases 0 and 32 (engine bases are 32-aligned)
                den_all = sb.tile([44, 256], F32, tag="den", bufs=2, name="den_all")
                rec_all = sb.tile([44, 256], F32, tag="den", bufs=2, name="rec_all")

                def den_slot(h, c):
                    return (0 if h < 6 else 32) + 2 * (h % 6) + c

                def ctx_norm_flush(h0):
                    base = 0 if h0 == 0 else 32
                    nc.vector.reciprocal(
                        rec_all[ds(base, 12), :], den_all[ds(base, 12), :]
                    )
                    # bounce via DRAM: partition-stride-0 reads are DRAM-only
                    rec_dr = dram.tile([12, 256], F32, tag="recdr", name="rec_dr")
                    nc.sync.dma_start(out=rec_dr[:], in_=rec_all[ds(base, 12), :])
                    for h_ in range(h0, h0 + 6):
                        kq_, po_ = h_ // 2, (h_ % 2) * 64
                        for c_ in range(2):
                            sl_ = 2 * (h_ - h0) + c_
                            recf = sb.tile([128, 256], F32, tag="recf", bufs=3, name="recf")
                            nc.sync.dma_start(
                                out=recf[ds(po_, 64), :],
                                in_=rec_dr[sl_ : sl_ + 1, :].to_broadcast((64, 256)),
                            )
                            nc.vector.tensor_tensor(
                                out=ctxT_sb[ds(po_, 64), kq_, ds(c_ * 256, 256)],
                                in0=ctxu_all[ds(po_, 64), kq_ % 3, c_, :],
                                in1=recf[ds(po_, 64), :],
                                op=AluOpType.mult,
                            )

                for kq in range(KT):
                    ets = {}
                    for rt in (2, 3, 4, 5, 0, 1, 6, 7):
                        c_m, slot = mask_slot[rt]
                        if rt in (0, 1):
                            qoff, nq = 0, 256
                        elif rt in (6, 7):
                            qoff, nq = 256, 256
                        else:
                            qoff, nq = 0, 512
                        moff = c_m * 256
                        # adjacent row-tiled MMs (bases 0 and 64) run concurrently
                        # on disjoint halves of the PE array
                        spp = ps_sps.tile([128, 2, 512], F32, tag="sps", name="spp")
                        for i in range(2):
                            nc.tensor.matmul(
                                spp[:, i, ds(qoff, nq)],
                                lhsT=kT_sb[ds(i * 64, 64), kq, ts(rt, 128)],
                                rhs=qT_sb[ds(i * 64, 64), kq, ds(qoff, nq)],
                                start=True, stop=True,
                            )
                        etp = sb.tile([128, 2, 512], BF16, tag="exp", bufs=10, name="etp")
                        # 256-col-per-bank ACT ops over both heads at once
                        for eo in range(qoff, qoff + nq, 256):
                            nc.scalar.activation(
                                etp[:, :, ds(eo, 256)], spp[:, :, ds(eo, 256)], AF.Exp
                            )
                        # zero invalid band positions (masks are 0/1 bf16); gpsimd
                        # is otherwise idle during attention
                        for i in range(2):
                            nc.gpsimd.tensor_tensor(
                                out=etp[:, i, ds(moff, 256)], in0=etp[:, i, ds(moff, 256)],
                                in1=masks_sb[:, c_m * 4 + slot, :], op=AluOpType.mult,
                            )
                        ets[rt] = etp
                    for i in range(2):
                        h, po = 2 * kq + i, i * 64
                        for c in range(2):
                            rts = (2, 3, 4, 5, 0, 1) if c == 0 else (2, 3, 4, 5, 6, 7)
                            cp = ps_ctx.tile([65, 256], F32, tag="ctx", name="cp")
                            for j, rt in enumerate(rts):
                                nc.tensor.matmul(
                                    cp[:],
                                    lhsT=v_buf[:, rt, ds(h * 65, 65)],
                                    rhs=ets[rt][:, i, ds(c * 256, 256)],
                                    start=(j == 0), stop=(j == 5),
                                )
                            nc.vector.tensor_copy(
                                ctxu_all[ds(po, 64), kq % 3, c, :], cp[0:64, :]
                            )
                            # den row -> SBUF (lane 64; engine partition bases must
                            # be 32-aligned), then DMA into the packed per-slot lane
                            den_sb = sb.tile([65, 256], F32, tag="densb", bufs=2, name="den_sb")
                            nc.vector.tensor_copy(den_sb[64:65, :], cp[64:65, :])
                            sl = den_slot(h, c)
                            nc.sync.dma_start(
                                out=den_all[sl : sl + 1, :], in_=den_sb[64:65, :]
                            )
                    if kq == 2:
                        ctx_norm_flush(0)
                ctx_norm_flush(6)

                # ---- Wo + residual + LN1
                wo_sb = load_wmat(wo, l, "wo_sb")
                xr = sb.tile([128, KT, OWN], F32, tag="xr", bufs=1, name="xr")
                for mt in range(KT):
                    op_ = ps_big.tile([128, OWN], F32, tag="big", name="op_")
                    for kt in range(KT):
                        nc.tensor.matmul(
                            op_[:], lhsT=wo_sb[:, kt, ts(mt, 128)],
                            rhs=ctxT_sb[:, kt, :],
                            start=(kt == 0), stop=(kt == KT - 1),
                        )
                    nc.vector.scalar_tensor_tensor(
                        out=xr[:, mt, :], in0=op_[:], scalar=bo_sb[:, mt : mt + 1],
                        in1=x_f_cur[:, mt, :], op0=AluOpType.add, op1=AluOpType.add,
                    )
                xp_bf = sb.tile([128, KT, OWN], BF16, tag="qT", bufs=1, name="xp_bf")
                ln_T(xr, ln1s_sb, ln1b_sb, None, xp_bf, 0)

                # ---- FFN1 (+gelu) -> h_bf
                h_lo = sb.tile([128, 12, OWN], BF16, tag="kT", bufs=1, name="h_lo")
                h_hi = sb.tile([128, 12, OWN], BF16, tag="hbuf", bufs=1, name="h_hi")

                def h_sl(mt):
                    return h_lo[:, mt, :] if mt < 12 else h_hi[:, mt - 12, :]
                for mtb in range(8):
                    w1t = sb.tile([128, KT, 384], BF16, tag="wf1t", bufs=2, name="w1t")
                    nc.sync.dma_start(
                        out=w1t[:],
                        in_=wf1.ap()[l].rearrange("(k p) m -> p k m", p=128)[
                            :, :, ds(mtb * 384, 384)
                        ],
                    )
                    for mi in range(3):
                        mt = mtb * 3 + mi
                        fp = ps_big.tile([128, OWN], F32, tag="big", name="fp")
                        for kt in range(KT):
                            nc.tensor.matmul(
                                fp[:], lhsT=w1t[:, kt, ts(mi, 128)],
                                rhs=xp_bf[:, kt, :],
                                start=(kt == 0), stop=(kt == KT - 1),
                            )
                        nc.scalar.activation(
                            h_sl(mt), fp[:], AF.Gelu_apprx_tanh,
                            bias=bf1_sb[:, mt : mt + 1],
                        )

                # ---- FFN2 + residual + LN2 -> next x
                xr2 = sb.tile([128, KT, OWN], F32, tag="xr", bufs=1, name="xr2")
                for mp in range(3):  # pairs of output tiles
                    f2ps = [
                        ps_big.tile([128, OWN], F32, tag="big", name=f"f2p{j}")
                        for j in range(2)
                    ]
                    for ktb in range(4):
                        w2t = sb.tile([128, KT, 256], BF16, tag="wf2t", bufs=2, name="w2t")
                        nc.sync.dma_start(
                            out=w2t[:],
                            in_=wf2.ap()[l].rearrange("(k p) m -> p k m", p=128)[
                                :, ds(ktb * KT, KT), ds(mp * 256, 256)
                            ],
                        )
                        for kt in range(KT):
                            for j in range(2):
                                nc.tensor.matmul(
                                    f2ps[j][:],
                                    lhsT=w2t[:, kt, ts(j, 128)],
                                    rhs=h_sl(ktb * KT + kt),
                                    start=(ktb == 0 and kt == 0),
                                    stop=(ktb == 3 and kt == KT - 1),
                                )
                    for j in range(2):
                        mt = mp * 2 + j
                        nc.vector.scalar_tensor_tensor(
                            out=xr2[:, mt, :], in0=f2ps[j][:],
                            scalar=bf2_sb[:, mt : mt + 1], in1=xp_bf[:, mt, :],
                            op0=AluOpType.add, op1=AluOpType.add,
                        )
                x_bf_next = sb.tile([128, KT, R], BF16, tag="xbf", bufs=1, name="x_bfn")
                x_f_next = sb.tile([128, KT, OWN], BF16, tag="xf", bufs=1, name="x_fn")
                ln_T(xr2, ln2s_sb, ln2b_sb, x_f_next, x_bf_next, 256)
                x_bf_cur, x_f_cur = x_bf_next, x_f_next

            # ================= LSTM / pooling tail =================
            send2 = dram.tile([D, OWN], BF16, tag="send", name="send2")
            nc.sync.dma_start(
                out=send2.rearrange("(k p) c -> p k c", p=128),
                in_=x_bf_cur[:, :, 256 : 256 + OWN],
            )
            gath2 = dram.tile([NC * D, OWN], BF16, addr_space="Shared", tag="gath", name="gath2")
            nc.gpsimd.collective_compute(
                "AllGather", AluOpType.bypass, ins=[send2.opt()], outs=[gath2.opt()],
                replica_groups=[list(range(NC))],
            )
            xbT = []
            for bi, (tag, sv) in enumerate((("qT", qD_sv), ("ctxT", q4D_sv))):
                t = sb.tile([128, KT, OWN], BF16, tag=tag, bufs=1, name=f"xb{bi}")
                nc.sync.dma_start(
                    out=t[:, :, :],
                    in_=gath2[ds(sv, D), :].rearrange("(k p) c -> p k c", p=128),
                )
                xbT.append(t)

            # LSTM1 weights (reuse big encoder tags, free by now)
            l1w_sb, l1h_sb, l1b_sb = [], [], []
            for d_, tag in ((0, "kT"), (1, "xbf")):
                wt = sb.tile([128, KT, 1024], BF16, tag=tag, bufs=1, name=f"l1w{d_}")
                nc.sync.dma_start(
                    out=wt[:], in_=l1wih.ap()[d_].rearrange("(k p) m -> p k m", p=128)
                )
                l1w_sb.append(wt)
                ht = sb.tile([128, 2, 1024], BF16, tag="wf1t", bufs=2, name=f"l1h{d_}")
                nc.sync.dma_start(
                    out=ht[:], in_=l1whh.ap()[d_].rearrange("(k p) m -> p k m", p=128)
                )
                l1h_sb.append(ht)
                b1 = sb.tile([128, 8], F32, tag="l1b", bufs=4, name=f"l1bi{d_}")
                nc.sync.dma_start(
                    out=b1[:], in_=l1bih.ap()[d_].rearrange("(k p) -> p k", p=128)
                )
                b2 = sb.tile([128, 8], F32, tag="l1b", bufs=4, name=f"l1bh{d_}")
                nc.sync.dma_start(
                    out=b2[:], in_=l1bhh.ap()[d_].rearrange("(k p) -> p k", p=128)
                )
                nc.vector.tensor_tensor(out=b1[:], in0=b1[:], in1=b2[:], op=AluOpType.add)
                l1b_sb.append(b1)

            GATE_F = {0: AF.Sigmoid, 1: AF.Sigmoid, 2: AF.Tanh, 3: AF.Sigmoid}

            def lstm_step(w_sb, h_sb, b_sb, x_in, nkt, h_prev_bf, c_prev, nn):
                """One LSTM step in T layout; returns (h_bf16, c_f32) [128,2,nn]."""
                gates = []
                for mt in range(8):
                    gp = ps_big.tile([128, OWN], F32, tag="big", name="gp")
                    for i in range(nkt):
                        nc.tensor.matmul(
                            gp[:, :nn], lhsT=w_sb[:, i, ts(mt, 128)],
                            rhs=x_in[:, i, :nn],
                            start=(i == 0),
                            stop=(h_prev_bf is None and i == nkt - 1),
                        )
                    if h_prev_bf is not None:
                        for i in range(2):
                            nc.tensor.matmul(
                                gp[:, :nn], lhsT=h_sb[:, i, ts(mt, 128)],
                                rhs=h_prev_bf[:, i, :nn],
                                start=False, stop=(i == 1),
                            )
                    g = t_exp_bf(f"g{mt}")
                    nc.scalar.activation(
                        g[:, :nn], gp[:, :nn], GATE_F[mt // 2],
                        bias=b_sb[:, mt : mt + 1],
                    )
                    gates.append(g)
                h_b = sb.tile([128, 2, OWN], BF16, tag="exp", bufs=10, name="h_b")
                c_f = sb.tile([128, 2, OWN], F32, tag="wqkvo", bufs=2, name="c_f")
                for i in range(2):
                    tg = sb.tile([128, OWN], F32, tag="wf2t", bufs=2, name="tg")
                    nc.vector.tensor_tensor(
                        out=tg[:, :nn], in0=gates[0 + i][:, :nn],
                        in1=gates[4 + i][:, :nn], op=AluOpType.mult,
                    )  # sig(i)*tanh(g)
                    if c_prev is not None:
                        t2 = sb.tile([128, OWN], F32, tag="wf2t", bufs=2, name="t2")
                        nc.vector.tensor_tensor(
                            out=t2[:, :nn], in0=gates[2 + i][:, :nn],
                            in1=c_prev[:, i, :nn], op=AluOpType.mult,
                        )  # sig(f)*c
                        nc.vector.tensor_tensor(
                            out=c_f[:, i, :nn], in0=tg[:, :nn], in1=t2[:, :nn],
                            op=AluOpType.add,
                        )
                    else:
                        nc.vector.tensor_copy(c_f[:, i, :nn], tg[:, :nn])
                    tc_ = sb.tile([128, OWN], F32, tag="wf2t", bufs=2, name="tc_")
                    nc.scalar.activation(tc_[:, :nn], c_f[:, i, :nn], AF.Tanh)
                    nc.vector.tensor_tensor(
                        out=h_b[:, i, :nn], in0=gates[6 + i][:, :nn],
                        in1=tc_[:, :nn], op=AluOpType.mult,
                    )
                return h_b, c_f

            fh0b, fc0 = lstm_step(l1w_sb[0], l1h_sb[0], l1b_sb[0], xbT[0], KT, None, None, OWN)
            fh1b, _ = lstm_step(l1w_sb[0], l1h_sb[0], l1b_sb[0], xbT[1], KT, fh0b, fc0, OWN)
            bh0b, bc0 = lstm_step(l1w_sb[1], l1h_sb[1], l1b_sb[1], xbT[1], KT, None, None, OWN)
            bh1b, _ = lstm_step(l1w_sb[1], l1h_sb[1], l1b_sb[1], xbT[0], KT, bh0b, bc0, OWN)

            # lstm1 output per t in T layout, as 4 slices [128, OWN] (kt = feat tile)
            def l1rhs(t_, kt):
                fh, bh = (fh0b, bh1b) if t_ == 0 else (fh1b, bh0b)
                return fh[:, kt, :] if kt < 2 else bh[:, kt - 2, :]

            # natural-layout copies via PE transpose: l1N[t_] [128(cols), 4, 512(feat)]
            l1N = []
            for t_, tag in ((0, "xrbf"), (1, "xsq")):
                ln_ = sb.tile([128, 4, OWN], BF16, tag=tag, bufs=1, name=f"l1N{t_}")
                for ft in range(4):
                    for ct in range(4):
                        tp2 = ps_sps.tile([128, 128], BF16, tag="sps", name="tp2")
                        nc.tensor.transpose(tp2[:], l1rhs(t_, ft)[:, ts(ct, 128)], ident[:])
                        nc.scalar.activation(ln_[:, ct, ts(ft, 128)], tp2[:], AF.Copy)
                l1N.append(ln_)

            # attention pooling partials
            paw1_sb = sb.tile([128, 4, 64], BF16, tag="paw1", bufs=1, name="paw1_sb")
            nc.sync.dma_start(
                out=paw1_sb[:], in_=paw1.ap().rearrange("(k p) m -> p k m", p=128)
            )
            paw2_sb = sb.tile([64, 1], BF16, tag="paw2", bufs=1, name="paw2_sb")
            nc.sync.dma_start(out=paw2_sb[:], in_=paw2.ap())
            pab1_sb = sb.tile([64, 1], F32, tag="pab1", bufs=1, name="pab1_sb")
            nc.sync.dma_start(out=pab1_sb[:], in_=pab1.ap())
            pab2_sb = sb.tile([1, 1], F32, tag="pab2", bufs=1, name="pab2_sb")
            nc.sync.dma_start(out=pab2_sb[:], in_=pab2.ap())
            pab2_full = sb.tile([128, 1], F32, tag="pab2f", bufs=1, name="pab2_full")
            nc.gpsimd.partition_broadcast(pab2_full[:], pab2_sb[:])

            st = sb.tile([128, 10], F32, tag="st", bufs=1, name="st")
            nc.vector.memset(st[:], 0.0)
            for t_ in range(2):
                rp = ps_sps.tile([64, OWN], F32, tag="sps", name="rp")
                for kt in range(4):
                    nc.tensor.matmul(
                        rp[:], lhsT=paw1_sb[:, kt, :], rhs=l1rhs(t_, kt),
                        start=(kt == 0), stop=(kt == 3),
                    )
                relu_bf = sb.tile([64, OWN], BF16, tag="relu", bufs=2, name="relu_bf")
                nc.scalar.activation(relu_bf[:], rp[:], AF.Relu, bias=pab1_sb[:, :1])
                wcol = sb.tile([128, 4, 1], BF16, tag="wcol", bufs=2, name="wcol")
                for ct in range(4):
                    ep = ps_sps.tile([128, 1], F32, tag="sps", name="ep")
                    nc.tensor.matmul(
                        ep[:], lhsT=relu_bf[:, ts(ct, 128)], rhs=paw2_sb[:],
                        start=True, stop=True,
                    )
                    etmp = sb.tile([128, 1], F32, tag="etmp", bufs=4, name="etmp")
                    nc.vector.tensor_tensor(
                        out=etmp[:], in0=ep[:], in1=pab2_full[:], op=AluOpType.add
                    )
                    nc.scalar.activation(wcol[:, ct, :], etmp[:], AF.Exp)
                for mt in range(4):
                    pn = ps_sps.tile([128, 1], F32, tag="sps", name="pn")
                    for ct in range(4):
                        nc.tensor.matmul(
                            pn[:], lhsT=l1N[t_][:, ct, ts(mt, 128)], rhs=wcol[:, ct, :],
                            start=(ct == 0), stop=(ct == 3),
                        )
                    nc.scalar.activation(
                        st[:, t_ * 5 + mt : t_ * 5 + mt + 1], pn[:], AF.Copy
                    )
                pd = ps_sps.tile([1, 1], F32, tag="sps", name="pd")
                for ct in range(4):
                    nc.tensor.matmul(
                        pd[:], lhsT=wcol[:, ct, :], rhs=ones_col[:],
                        start=(ct == 0), stop=(ct == 3),
                    )
                nc.scalar.activation(st[0:1, t_ * 5 + 4 : t_ * 5 + 5], pd[:], AF.Copy)

            ar_in = dram.tile([128, 10], F32, tag="arin", name="ar_in")
            nc.sync.dma_start(out=ar_in[:], in_=st[:])
            ar_out = dram.tile([128, 10], F32, addr_space="Shared", tag="arout", name="ar_out")
            nc.gpsimd.collective_compute(
                "AllReduce", AluOpType.add, ins=[ar_in.opt()], outs=[ar_out.opt()],
                replica_groups=[list(range(NC))],
            )
            ared = sb.tile([128, 10], F32, tag="ared", bufs=1, name="ared")
            nc.sync.dma_start(out=ared[:], in_=ar_out[:])

            pooledT = sb.tile([128, 4, 2], BF16, tag="pooledT", bufs=1, name="pooledT")
            for t_ in range(2):
                dn = sb.tile([1, 1], F32, tag="dn", bufs=4, name="dn")
                nc.vector.reciprocal(dn[:], ared[0:1, t_ * 5 + 4 : t_ * 5 + 5])
                dnb = sb.tile([128, 1], F32, tag="dnb", bufs=4, name="dnb")
                nc.gpsimd.partition_broadcast(dnb[:], dn[:])
                for mt in range(4):
                    nc.vector.tensor_tensor(
                        out=pooledT[:, mt, t_ : t_ + 1],
                        in0=ared[:, t_ * 5 + mt : t_ * 5 + mt + 1],
                        in1=dnb[:], op=AluOpType.mult,
                    )

            # LSTM2: T=2 (batch), N=1, input 512
            l2w_sb, l2h_sb, l2b_sb = [], [], []
            for d_, wtag_ in ((0, "xrbf"), (1, "xsq")):
                wt = sb.tile([128, 4, 1024], BF16, tag=wtag_, bufs=1, name=f"l2w{d_}")
                nc.sync.dma_start(
                    out=wt[:], in_=l2wih.ap()[d_].rearrange("(k p) m -> p k m", p=128)
                )
                l2w_sb.append(wt)
                ht = sb.tile([128, 2, 1024], BF16, tag="wf1t", bufs=2, name=f"l2h{d_}")
                nc.sync.dma_start(
                    out=ht[:], in_=l2whh.ap()[d_].rearrange("(k p) m -> p k m", p=128)
                )
                l2h_sb.append(ht)
                b1 = sb.tile([128, 8], F32, tag="l2b", bufs=4, name=f"l2bi{d_}")
                nc.sync.dma_start(
                    out=b1[:], in_=l2bih.ap()[d_].rearrange("(k p) -> p k", p=128)
                )
                b2 = sb.tile([128, 8], F32, tag="l2b", bufs=4, name=f"l2bh{d_}")
                nc.sync.dma_start(
                    out=b2[:], in_=l2bhh.ap()[d_].rearrange("(k p) -> p k", p=128)
                )
                nc.vector.tensor_tensor(out=b1[:], in0=b1[:], in1=b2[:], op=AluOpType.add)
                l2b_sb.append(b1)

            def l2_step(d_, t_src, h_prev_bf, c_prev):
                gates = []
                for mt in range(8):
                    gp2 = ps_sps.tile([128, 1], F32, tag="sps", name="gp2")
                    for kt in range(4):
                        nc.tensor.matmul(
                            gp2[:], lhsT=l2w_sb[d_][:, kt, ts(mt, 128)],
                            rhs=pooledT[:, kt, t_src : t_src + 1],
                            start=(kt == 0), stop=(h_prev_bf is None and kt == 3),
                        )
                    if h_prev_bf is not None:
                        for i in range(2):
                            nc.tensor.matmul(
                                gp2[:], lhsT=l2h_sb[d_][:, i, ts(mt, 128)],
                                rhs=h_prev_bf[:, i, :], start=False, stop=(i == 1),
                            )
                    g = sb.tile([128, 1], F32, tag="g2", bufs=10, name="g2")
                    nc.scalar.activation(
                        g[:], gp2[:], GATE_F[mt // 2], bias=l2b_sb[d_][:, mt : mt + 1]
                    )
                    gates.append(g)
                h_b = sb.tile([128, 2, 1], BF16, tag="hc2b", bufs=6, name="h2b")
                c_f = sb.tile([128, 2, 1], F32, tag="hc2", bufs=6, name="c2f")
                for i in range(2):
                    tg = sb.tile([128, 1], F32, tag="tg2", bufs=4, name="tg2")
                    nc.vector.tensor_tensor(
                        out=tg[:], in0=gates[0 + i][:, :1], in1=gates[4 + i][:, :1],
                        op=AluOpType.mult,
                    )
                    if c_prev is not None:
                        t2 = sb.tile([128, 1], F32, tag="tg2", bufs=4, name="t22")
                        nc.vector.tensor_tensor(
                            out=t2[:], in0=gates[2 + i][:, :1], in1=c_prev[:, i, :],
                            op=AluOpType.mult,
                        )
                        nc.vector.tensor_tensor(
                            out=c_f[:, i, :], in0=tg[:], in1=t2[:], op=AluOpType.add
                        )
                    else:
                        nc.vector.tensor_copy(c_f[:, i, :], tg[:])
                    tc_ = sb.tile([128, 1], F32, tag="tg2", bufs=4, name="tc2")
                    nc.scalar.activation(tc_[:], c_f[:, i, :], AF.Tanh)
                    nc.vector.tensor_tensor(
                        out=h_b[:, i, :], in0=gates[6 + i][:, :1], in1=tc_[:],
                        op=AluOpType.mult,
                    )
                return h_b, c_f

            f2h0b, f2c0 = l2_step(0, 0, None, None)
            f2h1b, _ = l2_step(0, 1, f2h0b, f2c0)
            b2h0b, b2c0 = l2_step(1, 1, None, None)
            b2h1b, _ = l2_step(1, 0, b2h0b, b2c0)

            # flatT [128, 4, 2]: t0 = [f2h0, b2h1], t1 = [f2h1, b2h0]
            flatT = sb.tile([128, 4, 2], BF16, tag="flatT", bufs=1, name="flatT")
            for t_, (fh, bh) in enumerate([(f2h0b, b2h1b), (f2h1b, b2h0b)]):
                for i in range(2):
                    nc.vector.tensor_copy(flatT[:, i, t_ : t_ + 1], fh[:, i, :])
                    nc.vector.tensor_copy(flatT[:, 2 + i, t_ : t_ + 1], bh[:, i, :])

            wtag_sb = sb.tile([128, 4, 1], BF16, tag="wtag", bufs=1, name="wtag_sb")
            nc.sync.dma_start(
                out=wtag_sb[:], in_=wtag.ap().rearrange("(k p) m -> p k m", p=128)
            )
            btag_sb = sb.tile([1, 1], F32, tag="btag", bufs=1, name="btag_sb")
            nc.sync.dma_start(out=btag_sb[:], in_=btag.ap())
            fo = ps_sps.tile([1, 2], F32, tag="sps", name="fo")
            for kt in range(4):
                nc.tensor.matmul(
                    fo[:], lhsT=wtag_sb[:, kt, :], rhs=flatT[:, kt, :],
                    start=(kt == 0), stop=(kt == 3),
                )
            out_sb = sb.tile([1, 2], F32, tag="outsb", bufs=1, name="out_sb")
            nc.scalar.activation(out_sb[:], fo[:], AF.Identity, bias=btag_sb[:1, :1])
            nc.sync.dma_start(out=out.ap(), in_=out_sb[:])

    nc.finalize()
    return nc


def prep_inputs(inputs, L=LMAX):
    """Host-side sharding/layout prep. Returns in_maps for 8 cores."""
    f32 = lambda a: np.ascontiguousarray(np.asarray(a), dtype=np.float32)
    bf = lambda a: np.ascontiguousarray(np.asarray(np.asarray(a), dtype=np.float32).astype(BF))

    sentences = np.asarray(inputs["sentences"], dtype=np.int32)
    shared = {
        "word_emb": bf(inputs["word_emb"]),
        "embs": f32(inputs["emb_ln_s"]).reshape(1, D),
        "embb": f32(inputs["emb_ln_b"]).reshape(1, D),
        "wq": bf(inputs["Wq"][:L]), "wk": bf(inputs["Wk"][:L]),
        "wv": bf(inputs["Wv"][:L]), "wo": bf(inputs["Wo"][:L]),
        "bq": f32(inputs["bq"][:L]), "bk": f32(inputs["bk"][:L]),
        "bo": f32(
            np.asarray(inputs["bo"][:L], dtype=np.float32)
            + np.einsum(
                "ld,ldo->lo",
                np.asarray(inputs["bv"][:L], dtype=np.float32),
                np.asarray(inputs["Wo"][:L], dtype=np.float32),
            )
        ),
        "ln1s": f32(inputs["ln1_s"][:L]), "ln1b": f32(inputs["ln1_b"][:L]),
        "ln2s": f32(inputs["ln2_s"][:L]), "ln2b": f32(inputs["ln2_b"][:L]),
        "wf1": bf(inputs["Wf1"][:L]), "bf1": f32(inputs["bf1"][:L]),
        "wf2": bf(inputs["Wf2"][:L]), "bf2": f32(inputs["bf2"][:L]),
        "l1wih": bf(np.transpose(np.asarray(inputs["l1_Wih"]), (0, 2, 1))),
        "l1whh": bf(np.transpose(np.asarray(inputs["l1_Whh"]), (0, 2, 1))),
        "l1bih": f32(inputs["l1_bih"]), "l1bhh": f32(inputs["l1_bhh"]),
        "paw1": bf(inputs["pa_W1"]), "pab1": f32(inputs["pa_b1"]).reshape(64, 1),
        "paw2": bf(inputs["pa_W2"]), "pab2": f32(inputs["pa_b2"]).reshape(1, 1),
        "l2wih": bf(np.transpose(np.asarray(inputs["l2_Wih"]), (0, 2, 1))),
        "l2whh": bf(np.transpose(np.asarray(inputs["l2_Whh"]), (0, 2, 1))),
        "l2bih": f32(inputs["l2_bih"]), "l2bhh": f32(inputs["l2_bhh"]),
        "wtag": bf(inputs["Wtag"]), "btag": f32(inputs["btag"]).reshape(1, 1),
    }
    pos_emb = f32(inputs["pos_emb"])

    in_maps = []
    for g in range(NC):
        b, q = g // 4, g % 4
        pos0 = 512 * q - 256
        rows = np.arange(R) + pos0
        rows_c = np.clip(rows, 0, S - 1)
        m = dict(shared)
        m["sent_idx"] = np.ascontiguousarray(
            sentences[b][rows_c].reshape(R, 1).astype(np.int32)
        )
        m["pos_sl"] = np.ascontiguousarray(pos_emb[rows_c])
        # masks [2 chunks, 4 slots, 128 keys(p), 256 queries(i)] — 0/1 validity
        mk = np.zeros((2, 4, 128, 256), dtype=np.float32)
        slot_wk = [0, 1, 4, 5]
        i_idx = np.arange(256)
        p_idx = np.arange(128)
        for c in range(2):
            qpos = 512 * q + 256 * c + i_idx  # [256] global query positions
            for s_, wk_ in enumerate(slot_wk):
                kpos = pos0 + 256 * c + wk_ * 128 + p_idx  # [128] global key pos
                valid = (
                    (np.abs(kpos[:, None] - qpos[None, :]) <= W)
                    & (kpos[:, None] >= 0)
                    & (kpos[:, None] < S)
                )
                mk[c, s_][valid] = 1.0
        m["masks"] = np.ascontiguousarray(mk.reshape(2 * 4 * 128, 256).astype(BF))
        # picks [4, KT, 128]: left-halo rank, right-halo rank, batch0 rank, batch1 rank
        d = np.arange(D)
        gl = g - 1 if q > 0 else g
        gr = g + 1 if q < 3 else g
        pk = np.stack(
            [gl * D + d, gr * D + d, q * D + d, (q + 4) * D + d]
        ).reshape(4, KT, 128)
        m["picks"] = np.ascontiguousarray(pk.reshape(4 * KT * 128, 1).astype(np.int32))
        in_maps.append(m)
    return in_maps


_NC_CACHE = {}


def run(inputs, L=LMAX, trace=False):
    if L not in _NC_CACHE:
        _NC_CACHE[L] = build_nc(L)
    nc = _NC_CACHE[L]
    in_maps = prep_inputs(inputs, L)
    res = run_bass_kernel_spmd(nc, in_maps, core_ids=list(range(NC)), trace=trace)
    out = np.asarray(res.results[0]["out"], dtype=np.float32).reshape(B, 1)
    return out, res


def kernel(**inputs) -> np.ndarray:
    out, _ = run(inputs, L=LMAX, trace=False)
    return out



# revision 44
# speedup vs baseline: 1.0960x; 1.0960x over previous
"""Distributed Trainium2 kernel for nn_ADUPredictor (12-layer sliding-window
encoder + BiLSTM/attention-pool tail), SPMD across 8 NeuronCores.

Sharding: cores 0-3 = batch 0, cores 4-7 = batch 1; core g owns the 512-token
sequence quarter q = g%4 plus 256-token halos on each side. Activations are
kept feature-major ("T layout", [feature, token]) so every projection uses the
weight as the stationary matmul operand. Per-layer halo exchange = AllGather of
each core's own x (bf16) + indirect-DMA picks of the two neighbor boundary
blocks. The BiLSTM/pool tail runs per seq-quarter (both batches) with one small
AllReduce for the global softmax pooling; LSTM2 + tag head are replicated.
"""

import sys

if "/opt/trn_rl_repo" not in sys.path:
    sys.path.insert(0, "/opt/trn_rl_repo")

import numpy as np
import ml_dtypes

import concourse.bass as bass
import concourse.mybir as mybir
import concourse.tile as tile
from concourse import bacc
from concourse.bass import ds, ts
from concourse.bass_utils import run_bass_kernel_spmd
from concourse.masks import make_identity
from concourse.alu_op_type import AluOpType
import bass_rust

F32 = mybir.dt.float32
F8 = mybir.dt.float8e4
DR = mybir.MatmulPerfMode.DoubleRow
W2SCALE = 16.0
BF16 = mybir.dt.float16
I32 = mybir.dt.int32
AF = mybir.ActivationFunctionType
AX = bass_rust.AxisListType

B, S = 2, 2048
D, H, DH, W, LMAX, FF = 768, 12, 64, 256, 12, 3072
VOCAB = 50265
HID, HALF = 512, 256
KT = D // 128          # 6 feature tiles
RT = 8                 # 1024 rows / 128
OWN = 512              # own rows per core
R = 1024               # own + halos
NC = 8
EPS = 1e-5
MASKVAL = -50.0

BF = np.float16


def build_nc(L=LMAX):
    nc = bacc.Bacc(None)

    def param(name, shape, dt):
        return nc.declare_dram_parameter(name, list(shape), dt, isOutput=False)

    sent_idx = param("sent_idx", [R, 1], I32)
    pos_sl = param("pos_sl", [R, D], F32)
    masks = param("masks", [2 * 4 * 128, 256], BF16)
    picks = param("picks", [4 * KT * 128, 1], I32)

    word_emb = param("word_emb", [VOCAB, D], BF16)
    embs = param("embs", [1, D], F32)
    embb = param("embb", [1, D], F32)
    wq = param("wq", [L, D, D], BF16)
    wk = param("wk", [L, D, D], BF16)
    wv = param("wv", [L, D, D], BF16)
    wo = param("wo", [L, D, D], BF16)
    bq = param("bq", [L, D], F32)
    bk = param("bk", [L, D], F32)
    bo = param("bo", [L, D], F32)
    ln1s = param("ln1s", [L, D], F32)
    ln1b = param("ln1b", [L, D], F32)
    ln2s = param("ln2s", [L, D], F32)
    ln2b = param("ln2b", [L, D], F32)
    wf1 = param("wf1", [L, D, FF], BF16)
    bf1 = param("bf1", [L, FF], F32)
    wf2 = param("wf2", [L, FF, D], BF16)
    bf2 = param("bf2", [L, D], F32)

    l1wih = param("l1wih", [2, D, 4 * HALF], BF16)     # transposed [in, 4*HALF]
    l1whh = param("l1whh", [2, HALF, 4 * HALF], BF16)
    l1bih = param("l1bih", [2, 4 * HALF], F32)
    l1bhh = param("l1bhh", [2, 4 * HALF], F32)
    paw1 = param("paw1", [HID, 64], BF16)
    pab1 = param("pab1", [64, 1], F32)
    paw2 = param("paw2", [64, 1], BF16)
    pab2 = param("pab2", [1, 1], F32)
    l2wih = param("l2wih", [2, HID, 4 * HALF], BF16)
    l2whh = param("l2whh", [2, HALF, 4 * HALF], BF16)
    l2bih = param("l2bih", [2, 4 * HALF], F32)
    l2bhh = param("l2bhh", [2, 4 * HALF], F32)
    wtag = param("wtag", [HID, 1], BF16)
    btag = param("btag", [1, 1], F32)

    out = nc.declare_dram_parameter("out", [1, 2], F32, isOutput=True)

    with tile.TileContext(nc) as tc:
        with (
            tc.tile_pool(name="pers", bufs=1) as pers,
            tc.tile_pool(name="sb", bufs=1) as sb,
            tc.tile_pool(name="ps_sps", bufs=2, space="PSUM") as ps_sps,
            tc.tile_pool(name="ps_ctx", bufs=2, space="PSUM") as ps_ctx,
            tc.tile_pool(name="ps_big", bufs=2, space="PSUM") as ps_big,
            tc.tile_pool(name="dram", bufs=2, space="DRAM") as dram,
        ):
            # ---------------- persistent constants ----------------
            ones_col = pers.tile([128, 1], BF16)   # lhsT for partition sums
            nc.vector.memset(ones_col[:], 1.0)
            ones_row_f = pers.tile([1, 128], F32)  # lhsT for PE row-broadcasts
            nc.vector.memset(ones_row_f[:], 1.0)
            ones_row_bf = pers.tile([1, 128], BF16)
            nc.vector.memset(ones_row_bf[:], 1.0)
            ident = pers.tile([128, 128], BF16)
            make_identity(nc, ident[:])

            v_buf = pers.tile([128, RT, H * 65], BF16)
            nc.vector.memset(
                v_buf[:].rearrange("p r (h e) -> p (r h) e", e=65)[:, :, 64:65], 1.0
            )

            masks_sb = pers.tile([128, 8, 256], BF16)
            nc.sync.dma_start(
                out=masks_sb[:],
                in_=masks.ap().rearrange("(m p) q -> p m q", p=128),
            )

            idx_sb = pers.tile([128, RT, 1], I32)
            nc.sync.dma_start(
                out=idx_sb[:], in_=sent_idx.ap().rearrange("(t p) o -> p t o", p=128)
            )
            eps_sb = pers.tile([128, 1], F32)
            nc.vector.memset(eps_sb[:], EPS)
            embs_row = pers.tile([1, D], F32)
            nc.sync.dma_start(out=embs_row[:], in_=embs.ap())
            embb_row = pers.tile([1, D], F32)
            nc.sync.dma_start(out=embb_row[:], in_=embb.ap())
            embs_row_bf = pers.tile([1, D], BF16)
            nc.vector.tensor_copy(embs_row_bf[:], embs_row[:])
            embb_row_bf = pers.tile([1, D], BF16)
            nc.vector.tensor_copy(embb_row_bf[:], embb_row[:])

            # scratch tag helpers (sizes chosen as max over all users)
            def t_exp(name):   # 2KB slot: exp tiles, lstm gates, emb gathers
                return sb.tile([128, 512], F32, tag="exp", bufs=10, name=name)

            def t_exp_bf(name):
                return sb.tile([128, 512], BF16, tag="exp", bufs=10, name=name)

            # ---------------- embedding (per row-tile, streaming) ----------------
            x_bf = sb.tile([128, KT, R], BF16, tag="xbf", bufs=1, name="x_bf0")
            x_f = sb.tile([128, KT, OWN], BF16, tag="xf", bufs=1, name="x_f0")
            embs_full = sb.tile([128, D], BF16, tag="lnt", bufs=4, name="embs_full")
            nc.gpsimd.partition_broadcast(embs_full[:], embs_row_bf[:])
            embb_full = sb.tile([128, D], BF16, tag="lnt", bufs=4, name="embb_full")
            nc.gpsimd.partition_broadcast(embb_full[:], embb_row_bf[:])
            for rt in range(RT):
                xg = sb.tile([128, D], BF16, tag="exp", bufs=10, name="xg")
                nc.gpsimd.indirect_dma_start(
                    out=xg[:],
                    out_offset=None,
                    in_=word_emb.ap(),
                    in_offset=bass.IndirectOffsetOnAxis(ap=idx_sb[:, rt, :], axis=0),
                )
                pos_rt = sb.tile([128, D], F32, tag="wqkvo", bufs=2, name="pos_rt")
                nc.sync.dma_start(out=pos_rt[:], in_=pos_sl.ap()[ts(rt, 128), :])
                xe = sb.tile([128, D], F32, tag="wf1t", bufs=2, name="xe")
                nc.vector.tensor_tensor(
                    out=xe[:], in0=xg[:], in1=pos_rt[:], op=AluOpType.add
                )
                # row-wise LN over free axis (features)
                m = sb.tile([128, 1], F32, tag="lnrow", bufs=8, name="m")
                nc.vector.reduce_sum(m[:], xe[:], axis=AX.X)
                nc.scalar.mul(m[:], m[:], 1.0 / D)
                xc = sb.tile([128, D], F32, tag="wf1t", bufs=2, name="xc")
                nc.vector.tensor_scalar_sub(xc[:], xe[:], m[:, :1])
                sq = sb.tile([128, D], F32, tag="wqkvo", bufs=2, name="sq")
                nc.scalar.activation(sq[:], xc[:], AF.Square)
                v_ = sb.tile([128, 1], F32, tag="lnrow", bufs=8, name="v_")
                nc.vector.reduce_sum(v_[:], sq[:], axis=AX.X)
                lnv = sb.tile([128, 1], F32, tag="lnrow", bufs=8, name="lnv")
                nc.scalar.activation(lnv[:], v_[:], AF.Ln, bias=eps_sb[:, :1], scale=1.0 / D)
                rstd = sb.tile([128, 1], F32, tag="lnrow", bufs=8, name="rstd")
                nc.scalar.activation(rstd[:], lnv[:], AF.Exp, scale=-0.5)
                nc.vector.tensor_scalar_mul(xc[:], xc[:], rstd[:, :1])
                nc.vector.tensor_tensor(out=xc[:], in0=xc[:], in1=embs_full[:], op=AluOpType.mult)
                xn_bf = sb.tile([128, D], BF16, tag="exp", bufs=10, name="xn_bf")
                nc.vector.tensor_tensor(
                    out=xn_bf[:], in0=xc[:], in1=embb_full[:], op=AluOpType.add
                )
                for kt in range(KT):
                    tp = ps_sps.tile([128, 128], BF16, tag="sps", name="tp")
                    nc.tensor.transpose(tp[:], xn_bf[:, ts(kt, 128)], ident[:])
                    nc.scalar.activation(x_bf[:, kt, ts(rt, 128)], tp[:], AF.Copy)
                    if 2 <= rt <= 5:
                        nc.vector.tensor_copy(x_f[:, kt, ts(rt - 2, 128)], tp[:])

            # ---------------- helpers ----------------
            def load_wmat(p, l, name):
                t = sb.tile([128, KT, D], BF16, tag="wqkvo", bufs=2, name=name)
                nc.sync.dma_start(
                    out=t[:], in_=p.ap()[l].rearrange("(k p) m -> p k m", p=128)
                )
                return t

            def load_vec(p, l, name, scale=None):
                t = sb.tile([128, KT], F32, tag="pvec", bufs=14, name=name)
                nc.sync.dma_start(
                    out=t[:], in_=p.ap()[l].rearrange("(k p) -> p k", p=128)
                )
                if scale is not None:
                    nc.scalar.mul(t[:], t[:], scale)
                return t

            def ln_T(xr, s_sb, b_sb, out_f, out_bf, out_bf_off):
                """LayerNorm over the partition (feature) axis of xr [128,KT,OWN].

                If out_f is not None writes f32 result there and a bf16 copy to
                out_bf; else writes bf16 result directly to out_bf."""
                xr_bf = sb.tile([128, KT, OWN], BF16, tag="xrbf", bufs=1, name="xr_bf")
                nc.vector.tensor_copy(xr_bf[:], xr[:])
                xsq_bf = sb.tile([128, KT, OWN], BF16, tag="xsq", bufs=1, name="xsq_bf")
                nc.scalar.activation(xsq_bf[:], xr[:], AF.Square)
                sum_ps = ps_sps.tile([1, OWN], F32, tag="sps", name="sum_ps")
                sq_ps = ps_sps.tile([1, OWN], F32, tag="sps", name="sq_ps")
                for kt in range(KT):
                    nc.tensor.matmul(
                        sum_ps[:], lhsT=ones_col[:], rhs=xr_bf[:, kt, :],
                        start=(kt == 0), stop=(kt == KT - 1),
                    )
                for kt in range(KT):
                    nc.tensor.matmul(
                        sq_ps[:], lhsT=ones_col[:], rhs=xsq_bf[:, kt, :],
                        start=(kt == 0), stop=(kt == KT - 1),
                    )
                m = sb.tile([1, OWN], F32, tag="lncol", bufs=6, name="m")
                nc.scalar.activation(m[:], sum_ps[:], AF.Copy, scale=1.0 / D)
                msq = sb.tile([1, OWN], F32, tag="lncol", bufs=6, name="msq")
                nc.vector.tensor_tensor(out=msq[:], in0=m[:], in1=m[:], op=AluOpType.mult)
                var = sb.tile([1, OWN], F32, tag="lncol", bufs=6, name="var")
                nc.vector.scalar_tensor_tensor(
                    out=var[:], in0=sq_ps[:], scalar=1.0 / D, in1=msq[:],
                    op0=AluOpType.mult, op1=AluOpType.subtract,
                )
                lnv = sb.tile([1, OWN], F32, tag="lncol", bufs=6, name="lnv")
                nc.scalar.activation(lnv[:], var[:], AF.Ln, bias=eps_sb[:1, :1])
                # bf16 rows -> PE rank-1 broadcast -> bf16 SBUF copies; the whole
                # apply then runs at DVE 2x/4x bf16 rates
                rstd_rb = sb.tile([1, OWN], BF16, tag="lncol", bufs=6, name="rstd_rb")
                nc.scalar.activation(rstd_rb[:], lnv[:], AF.Exp, scale=-0.5)
                mrs_rb = sb.tile([1, OWN], BF16, tag="lncol", bufs=6, name="mrs_rb")
                nc.vector.tensor_tensor(out=mrs_rb[:], in0=m[:], in1=rstd_rb[:], op=AluOpType.mult)
                rstd_ps = ps_ctx.tile([128, OWN], F32, tag="ctx", name="rstd_ps")
                nc.tensor.matmul(
                    rstd_ps[:], lhsT=ones_row_bf[:1, :], rhs=rstd_rb[:1, :],
                    start=True, stop=True,
                )
                mrs_ps = ps_ctx.tile([128, OWN], F32, tag="ctx", name="mrs_ps")
                nc.tensor.matmul(
                    mrs_ps[:], lhsT=ones_row_bf[:1, :], rhs=mrs_rb[:1, :],
                    start=True, stop=True,
                )
                rstd_fb = sb.tile([128, OWN], BF16, tag="lnt", bufs=4, name="rstd_fb")
                nc.scalar.activation(rstd_fb[:], rstd_ps[:], AF.Copy)
                mrs_fb = sb.tile([128, OWN], BF16, tag="lnt", bufs=4, name="mrs_fb")
                nc.scalar.activation(mrs_fb[:], mrs_ps[:], AF.Copy)
                for kt in range(KT):
                    t1 = sb.tile([128, OWN], BF16, tag="lnap", bufs=3, name="t1")
                    nc.vector.tensor_tensor(
                        out=t1[:], in0=xr_bf[:, kt, :], in1=rstd_fb[:], op=AluOpType.mult
                    )
                    nc.vector.tensor_tensor(
                        out=t1[:], in0=t1[:], in1=mrs_fb[:], op=AluOpType.subtract
                    )
                    nc.vector.tensor_scalar(
                        out=out_bf[:, kt, ds(out_bf_off, OWN)], in0=t1[:],
                        scalar1=s_sb[:, kt : kt + 1], scalar2=b_sb[:, kt : kt + 1],
                        op0=AluOpType.mult, op1=AluOpType.add,
                    )
                    if out_f is not None:
                        nc.vector.tensor_copy(
                            out_f[:, kt, :], out_bf[:, kt, ds(out_bf_off, OWN)]
                        )

            # masked rt map: rt -> (masked chunk, mask slot)
            mask_slot = {0: (0, 0), 1: (0, 1), 2: (1, 0), 3: (1, 1),
                         4: (0, 2), 5: (0, 3), 6: (1, 2), 7: (1, 3)}

            # ---- per-core halo source rows, computed in SP registers:
            # q = pid%4; gl = pid-1+((4-q)//4); gr = pid+1-((q+1)//4); offsets *D
            pid = nc.sync.partition_id()
            q_r = nc.sync.alloc_register("q_r")
            nc.sync.reg_mod(q_r, pid, 4)
            t0_r = nc.sync.alloc_register("t0_r")
            nc.sync.reg_sub(t0_r, 4, q_r)
            nc.sync.reg_div(t0_r, t0_r, 4)          # 1 iff q==0
            gl_r = nc.sync.alloc_register("gl_r")
            nc.sync.reg_add(gl_r, pid, t0_r)
            nc.sync.reg_sub(gl_r, gl_r, 1)
            nc.sync.reg_mul(gl_r, gl_r, D)
            t1_r = nc.sync.alloc_register("t1_r")
            nc.sync.reg_add(t1_r, q_r, 1)
            nc.sync.reg_div(t1_r, t1_r, 4)          # 1 iff q==3
            gr_r = nc.sync.alloc_register("gr_r")
            nc.sync.reg_sub(gr_r, pid, t1_r)
            nc.sync.reg_add(gr_r, gr_r, 1)
            nc.sync.reg_mul(gr_r, gr_r, D)
            glD_sv = nc.sync.snap(gl_r, min_val=0, max_val=(NC - 1) * D)
            grD_sv = nc.sync.snap(gr_r, min_val=0, max_val=(NC - 1) * D)
            # tail gathers: rows q*D (batch 0) and (q+4)*D (batch 1)
            qD_r = nc.sync.alloc_register("qD_r")
            nc.sync.reg_mul(qD_r, q_r, D)
            q4D_r = nc.sync.alloc_register("q4D_r")
            nc.sync.reg_add(q4D_r, qD_r, 4 * D)
            qD_sv = nc.sync.snap(qD_r, min_val=0, max_val=3 * D)
            q4D_sv = nc.sync.snap(q4D_r, min_val=4 * D, max_val=7 * D)

            x_bf_cur, x_f_cur = x_bf, x_f

            for l in range(L):
                wq_sb = load_wmat(wq, l, "wq_sb")
                wk_sb = load_wmat(wk, l, "wk_sb")
                bq8 = load_vec(bq, l, "bq8", scale=0.125)
                bk_sb = load_vec(bk, l, "bk_sb")
                bo_sb = load_vec(bo, l, "bo_sb")
                ln1s_sb = load_vec(ln1s, l, "ln1s_sb")
                ln1b_sb = load_vec(ln1b, l, "ln1b_sb")
                ln2s_sb = load_vec(ln2s, l, "ln2s_sb")
                ln2b_sb = load_vec(ln2b, l, "ln2b_sb")
                bf2_sb = load_vec(bf2, l, "bf2_sb")
                bf1_sb = sb.tile([128, FF // 128], F32, tag="bf1", bufs=2, name="bf1_sb")
                nc.sync.dma_start(
                    out=bf1_sb[:], in_=bf1.ap()[l].rearrange("(k p) -> p k", p=128)
                )

                # ---- halo exchange for this layer's x (l==0 computed locally)
                if l > 0:
                  with tc.high_priority():
                      send = dram.tile([D, OWN], BF16, tag="send", name="send")
                      nc.sync.dma_start(
                          out=send.rearrange("(k p) c -> p k c", p=128),
                          in_=x_bf_cur[:, :, 256 : 256 + OWN],
                      )
                      gathered = dram.tile(
                          [NC * D, OWN], BF16, addr_space="Shared", tag="gath", name="gathered"
                      )
                      nc.gpsimd.collective_compute(
                          "AllGather",
                          AluOpType.bypass,
                          ins=[send.opt()],
                          outs=[gathered.opt()],
                          replica_groups=[list(range(NC))],
                      )
                      nc.sync.dma_start(
                          out=x_bf_cur[:, :, 0:256],
                          in_=gathered[ds(glD_sv, D), ds(256, 256)].rearrange(
                              "(k p) c -> p k c", p=128
                          ),
                      )
                      nc.sync.dma_start(
                          out=x_bf_cur[:, :, 768:1024],
                          in_=gathered[ds(grD_sv, D), ds(0, 256)].rearrange(
                              "(k p) c -> p k c", p=128
                          ),
                      )

                # ---- qT (own rows), kT own
                qT_sb = sb.tile([128, KT, OWN], BF16, tag="qT", bufs=1, name="qT_sb")
                kT_sb = sb.tile([128, KT, R], BF16, tag="kT", bufs=1, name="kT_sb")
                for mt in range(KT):
                    qp = ps_big.tile([128, OWN], F32, tag="big", name="qp")
                    for kt in range(KT):
                        nc.tensor.matmul(
                            qp[:], lhsT=wq_sb[:, kt, ts(mt, 128)],
                            rhs=x_bf_cur[:, kt, 256 : 256 + OWN],
                            start=(kt == 0), stop=(kt == KT - 1),
                        )
                    nc.vector.tensor_scalar(
                        out=qT_sb[:, mt, :], in0=qp[:],
                        scalar1=bq8[:, mt : mt + 1], scalar2=0.125,
                        op0=AluOpType.add, op1=AluOpType.mult,
                    )
                    kp = ps_big.tile([128, OWN], F32, tag="big", name="kp")
                    for kt in range(KT):
                        nc.tensor.matmul(
                            kp[:], lhsT=wk_sb[:, kt, ts(mt, 128)],
                            rhs=x_bf_cur[:, kt, 256 : 256 + OWN],
                            start=(kt == 0), stop=(kt == KT - 1),
                        )
                    nc.vector.tensor_scalar_add(
                        kT_sb[:, mt, 256 : 256 + OWN], kp[:], bk_sb[:, mt : mt + 1]
                    )

                wv_sb = load_wmat(wv, l, "wv_sb")

                def v_rows(rt):
                    for nh in range(2):
                        vp = ps_big.tile([128, 384], F32, tag="big", name="vp")
                        for kt in range(KT):
                            nc.tensor.matmul(
                                vp[:], lhsT=x_bf_cur[:, kt, ts(rt, 128)],
                                rhs=wv_sb[:, kt, ts(nh, 384)],
                                start=(kt == 0), stop=(kt == KT - 1),
                            )
                        nc.scalar.activation(
                            v_buf[:, rt, :].rearrange("p (h e) -> p h e", e=65)[
                                :, ds(nh * 6, 6), 0:64
                            ],
                            vp[:].rearrange("p (h e) -> p h e", e=64),
                            AF.Copy,
                        )

                for rt in (2, 3, 4, 5):
                    v_rows(rt)

                # ---- halo kT, v (depend on picks)
                for mt in range(KT):
                    for side in range(2):
                        hoff = 0 if side == 0 else 768
                        kp2 = ps_big.tile([128, 256], F32, tag="big", name="kp2")
                        for kt in range(KT):
                            nc.tensor.matmul(
                                kp2[:], lhsT=wk_sb[:, kt, ts(mt, 128)],
                                rhs=x_bf_cur[:, kt, ds(hoff, 256)],
                                start=(kt == 0), stop=(kt == KT - 1),
                            )
                        nc.vector.tensor_scalar_add(
                            kT_sb[:, mt, ds(hoff if side == 0 else 768, 256)], kp2[:],
                            bk_sb[:, mt : mt + 1],
                        )
                for rt in (0, 1, 6, 7):
                    v_rows(rt)

                # ---- attention per head: scores+exp (8 rts) then ctx (2 chunks)
                ctxT_sb = sb.tile([128, KT, OWN], BF16, tag="ctxT", bufs=1, name="ctxT_sb")
                # unnormalized ctx stash (2 heads packed per partition group, slots
                # reused per 6-head half) + packed softmax denominators; reciprocal
                # runs batched over 12 (head, chunk) rows at a time
                ctxu_all = sb.tile([128, 3, 2, 256], F32, tag="ctxu", bufs=1, name="ctxu_all")
                # halves live at partition bases 0 and 32 (engine bases are 32-aligned)
                den_all = sb.tile([44, 256], F32, tag="den", bufs=2, name="den_all")
                rec_all = sb.tile([44, 256], F32, tag="den", bufs=2, name="rec_all")

                def den_slot(h, c):
                    return (0 if h < 6 else 32) + 2 * (h % 6) + c

                def ctx_norm_flush(h0):
                    base = 0 if h0 == 0 else 32
                    nc.vector.reciprocal(
                        rec_all[ds(base, 12), :], den_all[ds(base, 12), :]
                    )
                    # bounce via DRAM: partition-stride-0 reads are DRAM-only
                    rec_dr = dram.tile([12, 256], F32, tag="recdr", name="rec_dr")
                    nc.sync.dma_start(out=rec_dr[:], in_=rec_all[ds(base, 12), :])
                    for h_ in range(h0, h0 + 6):
                        kq_, po_ = h_ // 2, (h_ % 2) * 64
                        for c_ in range(2):
                            sl_ = 2 * (h_ - h0) + c_
                            recf = sb.tile([128, 256], F32, tag="recf", bufs=3, name="recf")
                            nc.sync.dma_start(
                                out=recf[ds(po_, 64), :],
                                in_=rec_dr[sl_ : sl_ + 1, :].to_broadcast((64, 256)),
                            )
                            nc.vector.tensor_tensor(
                                out=ctxT_sb[ds(po_, 64), kq_, ds(c_ * 256, 256)],
                                in0=ctxu_all[ds(po_, 64), kq_ % 3, c_, :],
                                in1=recf[ds(po_, 64), :],
                                op=AluOpType.mult,
                            )

                for kq in range(KT):
                    ets = {}
                    for rt in (2, 3, 4, 5, 0, 1, 6, 7):
                        c_m, slot = mask_slot[rt]
                        if rt in (0, 1):
                            qoff, nq = 0, 256
                        elif rt in (6, 7):
                            qoff, nq = 256, 256
                        else:
                            qoff, nq = 0, 512
                        moff = c_m * 256
                        # adjacent row-tiled MMs (bases 0 and 64) run concurrently
                        # on disjoint halves of the PE array
                        spp = ps_sps.tile([128, 2, 512], F32, tag="sps", name="spp")
                        for i in range(2):
                            nc.tensor.matmul(
                                spp[:, i, ds(qoff, nq)],
                                lhsT=kT_sb[ds(i * 64, 64), kq, ts(rt, 128)],
                                rhs=qT_sb[ds(i * 64, 64), kq, ds(qoff, nq)],
                                start=True, stop=True,
                            )
                        etp = sb.tile([128, 2, 512], BF16, tag="exp", bufs=10, name="etp")
                        # 256-col-per-bank ACT ops over both heads at once
                        for eo in range(qoff, qoff + nq, 256):
                            nc.scalar.activation(
                                etp[:, :, ds(eo, 256)], spp[:, :, ds(eo, 256)], AF.Exp
                            )
                        # zero invalid band positions (masks are 0/1 bf16)
                        for i in range(2):
                            nc.vector.tensor_tensor(
                                out=etp[:, i, ds(moff, 256)], in0=etp[:, i, ds(moff, 256)],
                                in1=masks_sb[:, c_m * 4 + slot, :], op=AluOpType.mult,
                            )
                        ets[rt] = etp
                    for i in range(2):
                        h, po = 2 * kq + i, i * 64
                        for c in range(2):
                            rts = (2, 3, 4, 5, 0, 1) if c == 0 else (2, 3, 4, 5, 6, 7)
                            cp = ps_ctx.tile([65, 256], F32, tag="ctx", name="cp")
                            for j, rt in enumerate(rts):
                                nc.tensor.matmul(
                                    cp[:],
                                    lhsT=v_buf[:, rt, ds(h * 65, 65)],
                                    rhs=ets[rt][:, i, ds(c * 256, 256)],
                                    start=(j == 0), stop=(j == 5),
                                )
                            nc.vector.tensor_copy(
                                ctxu_all[ds(po, 64), kq % 3, c, :], cp[0:64, :]
                            )
                            # den row -> SBUF (lane 64; engine partition bases must
                            # be 32-aligned), then DMA into the packed per-slot lane
                            den_sb = sb.tile([65, 256], F32, tag="densb", bufs=2, name="den_sb")
                            nc.vector.tensor_copy(den_sb[64:65, :], cp[64:65, :])
                            sl = den_slot(h, c)
                            nc.sync.dma_start(
                                out=den_all[sl : sl + 1, :], in_=den_sb[64:65, :]
                            )
                    if kq == 2:
                        ctx_norm_flush(0)
                ctx_norm_flush(6)

                # ---- Wo + residual + LN1
                wo_sb = load_wmat(wo, l, "wo_sb")
                xr = sb.tile([128, KT, OWN], F32, tag="xr", bufs=1, name="xr")
                for mt in range(KT):
                    op_ = ps_big.tile([128, OWN], F32, tag="big", name="op_")
                    for kt in range(KT):
                        nc.tensor.matmul(
                            op_[:], lhsT=wo_sb[:, kt, ts(mt, 128)],
                            rhs=ctxT_sb[:, kt, :],
                            start=(kt == 0), stop=(kt == KT - 1),
                        )
                    nc.vector.scalar_tensor_tensor(
                        out=xr[:, mt, :], in0=op_[:], scalar=bo_sb[:, mt : mt + 1],
                        in1=x_f_cur[:, mt, :], op0=AluOpType.add, op1=AluOpType.add,
                    )
                xp_bf = sb.tile([128, KT, OWN], BF16, tag="qT", bufs=1, name="xp_bf")
                ln_T(xr, ln1s_sb, ln1b_sb, None, xp_bf, 0)

                # ---- FFN1 (+gelu) -> h_bf
                h_lo = sb.tile([128, 12, OWN], BF16, tag="kT", bufs=1, name="h_lo")
                h_hi = sb.tile([128, 12, OWN], BF16, tag="hbuf", bufs=1, name="h_hi")

                def h_sl(mt):
                    return h_lo[:, mt, :] if mt < 12 else h_hi[:, mt - 12, :]
                for mtb in range(8):
                    w1t = sb.tile([128, KT, 384], BF16, tag="wf1t", bufs=2, name="w1t")
                    nc.sync.dma_start(
                        out=w1t[:],
                        in_=wf1.ap()[l].rearrange("(k p) m -> p k m", p=128)[
                            :, :, ds(mtb * 384, 384)
                        ],
                    )
                    for mi in range(3):
                        mt = mtb * 3 + mi
                        fp = ps_big.tile([128, OWN], F32, tag="big", name="fp")
                        for kt in range(KT):
                            nc.tensor.matmul(
                                fp[:], lhsT=w1t[:, kt, ts(mi, 128)],
                                rhs=xp_bf[:, kt, :],
                                start=(kt == 0), stop=(kt == KT - 1),
                            )
                        nc.scalar.activation(
                            h_sl(mt), fp[:], AF.Gelu_apprx_tanh,
                            bias=bf1_sb[:, mt : mt + 1],
                        )

                # ---- FFN2 + residual + LN2 -> next x
                xr2 = sb.tile([128, KT, OWN], F32, tag="xr", bufs=1, name="xr2")
                for mp in range(3):  # pairs of output tiles
                    f2ps = [
                        ps_big.tile([128, OWN], F32, tag="big", name=f"f2p{j}")
                        for j in range(2)
                    ]
                    for ktb in range(4):
                        w2t = sb.tile([128, KT, 256], BF16, tag="wf2t", bufs=2, name="w2t")
                        nc.sync.dma_start(
                            out=w2t[:],
                            in_=wf2.ap()[l].rearrange("(k p) m -> p k m", p=128)[
                                :, ds(ktb * KT, KT), ds(mp * 256, 256)
                            ],
                        )
                        for kt in range(KT):
                            for j in range(2):
                                nc.tensor.matmul(
                                    f2ps[j][:],
                                    lhsT=w2t[:, kt, ts(j, 128)],
                                    rhs=h_sl(ktb * KT + kt),
                                    start=(ktb == 0 and kt == 0),
                                    stop=(ktb == 3 and kt == KT - 1),
                                )
                    for j in range(2):
                        mt = mp * 2 + j
                        nc.vector.scalar_tensor_tensor(
                            out=xr2[:, mt, :], in0=f2ps[j][:],
                            scalar=bf2_sb[:, mt : mt + 1], in1=xp_bf[:, mt, :],
                            op0=AluOpType.add, op1=AluOpType.add,
                        )
                x_bf_next = sb.tile([128, KT, R], BF16, tag="xbf", bufs=1, name="x_bfn")
                x_f_next = sb.tile([128, KT, OWN], BF16, tag="xf", bufs=1, name="x_fn")
                ln_T(xr2, ln2s_sb, ln2b_sb, x_f_next, x_bf_next, 256)
                x_bf_cur, x_f_cur = x_bf_next, x_f_next

            # ================= LSTM / pooling tail =================
            send2 = dram.tile([D, OWN], BF16, tag="send", name="send2")
            nc.sync.dma_start(
                out=send2.rearrange("(k p) c -> p k c", p=128),
                in_=x_bf_cur[:, :, 256 : 256 + OWN],
            )
            gath2 = dram.tile([NC * D, OWN], BF16, addr_space="Shared", tag="gath", name="gath2")
            nc.gpsimd.collective_compute(
                "AllGather", AluOpType.bypass, ins=[send2.opt()], outs=[gath2.opt()],
                replica_groups=[list(range(NC))],
            )
            xbT = []
            for bi, (tag, sv) in enumerate((("qT", qD_sv), ("ctxT", q4D_sv))):
                t = sb.tile([128, KT, OWN], BF16, tag=tag, bufs=1, name=f"xb{bi}")
                nc.sync.dma_start(
                    out=t[:, :, :],
                    in_=gath2[ds(sv, D), :].rearrange("(k p) c -> p k c", p=128),
                )
                xbT.append(t)

            # LSTM1 weights (reuse big encoder tags, free by now)
            l1w_sb, l1h_sb, l1b_sb = [], [], []
            for d_, tag in ((0, "kT"), (1, "xbf")):
                wt = sb.tile([128, KT, 1024], BF16, tag=tag, bufs=1, name=f"l1w{d_}")
                nc.sync.dma_start(
                    out=wt[:], in_=l1wih.ap()[d_].rearrange("(k p) m -> p k m", p=128)
                )
                l1w_sb.append(wt)
                ht = sb.tile([128, 2, 1024], BF16, tag="wf1t", bufs=2, name=f"l1h{d_}")
                nc.sync.dma_start(
                    out=ht[:], in_=l1whh.ap()[d_].rearrange("(k p) m -> p k m", p=128)
                )
                l1h_sb.append(ht)
                b1 = sb.tile([128, 8], F32, tag="l1b", bufs=4, name=f"l1bi{d_}")
                nc.sync.dma_start(
                    out=b1[:], in_=l1bih.ap()[d_].rearrange("(k p) -> p k", p=128)
                )
                b2 = sb.tile([128, 8], F32, tag="l1b", bufs=4, name=f"l1bh{d_}")
                nc.sync.dma_start(
                    out=b2[:], in_=l1bhh.ap()[d_].rearrange("(k p) -> p k", p=128)
                )
                nc.vector.tensor_tensor(out=b1[:], in0=b1[:], in1=b2[:], op=AluOpType.add)
                l1b_sb.append(b1)

            GATE_F = {0: AF.Sigmoid, 1: AF.Sigmoid, 2: AF.Tanh, 3: AF.Sigmoid}

            def lstm_step(w_sb, h_sb, b_sb, x_in, nkt, h_prev_bf, c_prev, nn):
                """One LSTM step in T layout; returns (h_bf16, c_f32) [128,2,nn]."""
                gates = []
                for mt in range(8):
                    gp = ps_big.tile([128, OWN], F32, tag="big", name="gp")
                    for i in range(nkt):
                        nc.tensor.matmul(
                            gp[:, :nn], lhsT=w_sb[:, i, ts(mt, 128)],
                            rhs=x_in[:, i, :nn],
                            start=(i == 0),
                            stop=(h_prev_bf is None and i == nkt - 1),
                        )
                    if h_prev_bf is not None:
                        for i in range(2):
                            nc.tensor.matmul(
                                gp[:, :nn], lhsT=h_sb[:, i, ts(mt, 128)],
                                rhs=h_prev_bf[:, i, :nn],
                                start=False, stop=(i == 1),
                            )
                    g = t_exp_bf(f"g{mt}")
                    nc.scalar.activation(
                        g[:, :nn], gp[:, :nn], GATE_F[mt // 2],
                        bias=b_sb[:, mt : mt + 1],
                    )
                    gates.append(g)
                h_b = sb.tile([128, 2, OWN], BF16, tag="exp", bufs=10, name="h_b")
                c_f = sb.tile([128, 2, OWN], F32, tag="wqkvo", bufs=2, name="c_f")
                for i in range(2):
                    tg = sb.tile([128, OWN], F32, tag="wf2t", bufs=2, name="tg")
                    nc.vector.tensor_tensor(
                        out=tg[:, :nn], in0=gates[0 + i][:, :nn],
                        in1=gates[4 + i][:, :nn], op=AluOpType.mult,
                    )  # sig(i)*tanh(g)
                    if c_prev is not None:
                        t2 = sb.tile([128, OWN], F32, tag="wf2t", bufs=2, name="t2")
                        nc.vector.tensor_tensor(
                            out=t2[:, :nn], in0=gates[2 + i][:, :nn],
                            in1=c_prev[:, i, :nn], op=AluOpType.mult,
                        )  # sig(f)*c
                        nc.vector.tensor_tensor(
                            out=c_f[:, i, :nn], in0=tg[:, :nn], in1=t2[:, :nn],
                            op=AluOpType.add,
                        )
                    else:
                        nc.vector.tensor_copy(c_f[:, i, :nn], tg[:, :nn])
                    tc_ = sb.tile([128, OWN], F32, tag="wf2t", bufs=2, name="tc_")
                    nc.scalar.activation(tc_[:, :nn], c_f[:, i, :nn], AF.Tanh)
                    nc.vector.tensor_tensor(
                        out=h_b[:, i, :nn], in0=gates[6 + i][:, :nn],
                        in1=tc_[:, :nn], op=AluOpType.mult,
                    )
                return h_b, c_f

            fh0b, fc0 = lstm_step(l1w_sb[0], l1h_sb[0], l1b_sb[0], xbT[0], KT, None, None, OWN)
            fh1b, _ = lstm_step(l1w_sb[0], l1h_sb[0], l1b_sb[0], xbT[1], KT, fh0b, fc0, OWN)
            bh0b, bc0 = lstm_step(l1w_sb[1], l1h_sb[1], l1b_sb[1], xbT[1], KT, None, None, OWN)
            bh1b, _ = lstm_step(l1w_sb[1], l1h_sb[1], l1b_sb[1], xbT[0], KT, bh0b, bc0, OWN)

            # lstm1 output per t in T layout, as 4 slices [128, OWN] (kt = feat tile)
            def l1rhs(t_, kt):
                fh, bh = (fh0b, bh1b) if t_ == 0 else (fh1b, bh0b)
                return fh[:, kt, :] if kt < 2 else bh[:, kt - 2, :]

            # natural-layout copies via PE transpose: l1N[t_] [128(cols), 4, 512(feat)]
            l1N = []
            for t_, tag in ((0, "xrbf"), (1, "xsq")):
                ln_ = sb.tile([128, 4, OWN], BF16, tag=tag, bufs=1, name=f"l1N{t_}")
                for ft in range(4):
                    for ct in range(4):
                        tp2 = ps_sps.tile([128, 128], BF16, tag="sps", name="tp2")
                        nc.tensor.transpose(tp2[:], l1rhs(t_, ft)[:, ts(ct, 128)], ident[:])
                        nc.scalar.activation(ln_[:, ct, ts(ft, 128)], tp2[:], AF.Copy)
                l1N.append(ln_)

            # attention pooling partials
            paw1_sb = sb.tile([128, 4, 64], BF16, tag="paw1", bufs=1, name="paw1_sb")
            nc.sync.dma_start(
                out=paw1_sb[:], in_=paw1.ap().rearrange("(k p) m -> p k m", p=128)
            )
            paw2_sb = sb.tile([64, 1], BF16, tag="paw2", bufs=1, name="paw2_sb")
            nc.sync.dma_start(out=paw2_sb[:], in_=paw2.ap())
            pab1_sb = sb.tile([64, 1], F32, tag="pab1", bufs=1, name="pab1_sb")
            nc.sync.dma_start(out=pab1_sb[:], in_=pab1.ap())
            pab2_sb = sb.tile([1, 1], F32, tag="pab2", bufs=1, name="pab2_sb")
            nc.sync.dma_start(out=pab2_sb[:], in_=pab2.ap())
            pab2_full = sb.tile([128, 1], F32, tag="pab2f", bufs=1, name="pab2_full")
            nc.gpsimd.partition_broadcast(pab2_full[:], pab2_sb[:])

            st = sb.tile([128, 10], F32, tag="st", bufs=1, name="st")
            nc.vector.memset(st[:], 0.0)
            for t_ in range(2):
                rp = ps_sps.tile([64, OWN], F32, tag="sps", name="rp")
                for kt in range(4):
                    nc.tensor.matmul(
                        rp[:], lhsT=paw1_sb[:, kt, :], rhs=l1rhs(t_, kt),
                        start=(kt == 0), stop=(kt == 3),
                    )
                relu_bf = sb.tile([64, OWN], BF16, tag="relu", bufs=2, name="relu_bf")
                nc.scalar.activation(relu_bf[:], rp[:], AF.Relu, bias=pab1_sb[:, :1])
                wcol = sb.tile([128, 4, 1], BF16, tag="wcol", bufs=2, name="wcol")
                for ct in range(4):
                    ep = ps_sps.tile([128, 1], F32, tag="sps", name="ep")
                    nc.tensor.matmul(
                        ep[:], lhsT=relu_bf[:, ts(ct, 128)], rhs=paw2_sb[:],
                        start=True, stop=True,
                    )
                    etmp = sb.tile([128, 1], F32, tag="etmp", bufs=4, name="etmp")
                    nc.vector.tensor_tensor(
                        out=etmp[:], in0=ep[:], in1=pab2_full[:], op=AluOpType.add
                    )
                    nc.scalar.activation(wcol[:, ct, :], etmp[:], AF.Exp)
                for mt in range(4):
                    pn = ps_sps.tile([128, 1], F32, tag="sps", name="pn")
                    for ct in range(4):
                        nc.tensor.matmul(
                            pn[:], lhsT=l1N[t_][:, ct, ts(mt, 128)], rhs=wcol[:, ct, :],
                            start=(ct == 0), stop=(ct == 3),
                        )
                    nc.scalar.activation(
                        st[:, t_ * 5 + mt : t_ * 5 + mt + 1], pn[:], AF.Copy
                    )
                pd = ps_sps.tile([1, 1], F32, tag="sps", name="pd")
                for ct in range(4):
                    nc.tensor.matmul(
                        pd[:], lhsT=wcol[:, ct, :], rhs=ones_col[:],
                        start=(ct == 0), stop=(ct == 3),
                    )
                nc.scalar.activation(st[0:1, t_ * 5 + 4 : t_ * 5 + 5], pd[:], AF.Copy)

            ar_in = dram.tile([128, 10], F32, tag="arin", name="ar_in")
            nc.sync.dma_start(out=ar_in[:], in_=st[:])
            ar_out = dram.tile([128, 10], F32, addr_space="Shared", tag="arout", name="ar_out")
            nc.gpsimd.collective_compute(
                "AllReduce", AluOpType.add, ins=[ar_in.opt()], outs=[ar_out.opt()],
                replica_groups=[list(range(NC))],
            )
            ared = sb.tile([128, 10], F32, tag="ared", bufs=1, name="ared")
            nc.sync.dma_start(out=ared[:], in_=ar_out[:])

            pooledT = sb.tile([128, 4, 2], BF16, tag="pooledT", bufs=1, name="pooledT")
            for t_ in range(2):
                dn = sb.tile([1, 1], F32, tag="dn", bufs=4, name="dn")
                nc.vector.reciprocal(dn[:], ared[0:1, t_ * 5 + 4 : t_ * 5 + 5])
                dnb = sb.tile([128, 1], F32, tag="dnb", bufs=4, name="dnb")
                nc.gpsimd.partition_broadcast(dnb[:], dn[:])
                for mt in range(4):
                    nc.vector.tensor_tensor(
                        out=pooledT[:, mt, t_ : t_ + 1],
                        in0=ared[:, t_ * 5 + mt : t_ * 5 + mt + 1],
                        in1=dnb[:], op=AluOpType.mult,
                    )

            # LSTM2: T=2 (batch), N=1, input 512
            l2w_sb, l2h_sb, l2b_sb = [], [], []
            for d_, wtag_ in ((0, "xrbf"), (1, "xsq")):
                wt = sb.tile([128, 4, 1024], BF16, tag=wtag_, bufs=1, name=f"l2w{d_}")
                nc.sync.dma_start(
                    out=wt[:], in_=l2wih.ap()[d_].rearrange("(k p) m -> p k m", p=128)
                )
                l2w_sb.append(wt)
                ht = sb.tile([128, 2, 1024], BF16, tag="wf1t", bufs=2, name=f"l2h{d_}")
                nc.sync.dma_start(
                    out=ht[:], in_=l2whh.ap()[d_].rearrange("(k p) m -> p k m", p=128)
                )
                l2h_sb.append(ht)
                b1 = sb.tile([128, 8], F32, tag="l2b", bufs=4, name=f"l2bi{d_}")
                nc.sync.dma_start(
                    out=b1[:], in_=l2bih.ap()[d_].rearrange("(k p) -> p k", p=128)
                )
                b2 = sb.tile([128, 8], F32, tag="l2b", bufs=4, name=f"l2bh{d_}")
                nc.sync.dma_start(
                    out=b2[:], in_=l2bhh.ap()[d_].rearrange("(k p) -> p k", p=128)
                )
                nc.vector.tensor_tensor(out=b1[:], in0=b1[:], in1=b2[:], op=AluOpType.add)
                l2b_sb.append(b1)

            def l2_step(d_, t_src, h_prev_bf, c_prev):
                gates = []
                for mt in range(8):
                    gp2 = ps_sps.tile([128, 1], F32, tag="sps", name="gp2")
                    for kt in range(4):
                        nc.tensor.matmul(
                            gp2[:], lhsT=l2w_sb[d_][:, kt, ts(mt, 128)],
                            rhs=pooledT[:, kt, t_src : t_src + 1],
                            start=(kt == 0), stop=(h_prev_bf is None and kt == 3),
                        )
                    if h_prev_bf is not None:
                        for i in range(2):
                            nc.tensor.matmul(
                                gp2[:], lhsT=l2h_sb[d_][:, i, ts(mt, 128)],
                                rhs=h_prev_bf[:, i, :], start=False, stop=(i == 1),
                            )
                    g = sb.tile([128, 1], F32, tag="g2", bufs=10, name="g2")
                    nc.scalar.activation(
                        g[:], gp2[:], GATE_F[mt // 2], bias=l2b_sb[d_][:, mt : mt + 1]
                    )
                    gates.append(g)
                h_b = sb.tile([128, 2, 1], BF16, tag="hc2b", bufs=6, name="h2b")
                c_f = sb.tile([128, 2, 1], F32, tag="hc2", bufs=6, name="c2f")
                for i in range(2):
                    tg = sb.tile([128, 1], F32, tag="tg2", bufs=4, name="tg2")
                    nc.vector.tensor_tensor(
                        out=tg[:], in0=gates[0 + i][:, :1], in1=gates[4 + i][:, :1],
                        op=AluOpType.mult,
                    )
                    if c_prev is not None:
                        t2 = sb.tile([128, 1], F32, tag="tg2", bufs=4, name="t22")
                        nc.vector.tensor_tensor(
                            out=t2[:], in0=gates[2 + i][:, :1], in1=c_prev[:, i, :],
                            op=AluOpType.mult,
                        )
                        nc.vector.tensor_tensor(
                            out=c_f[:, i, :], in0=tg[:], in1=t2[:], op=AluOpType.add
                        )
                    else:
                        nc.vector.tensor_copy(c_f[:, i, :], tg[:])
                    tc_ = sb.tile([128, 1], F32, tag="tg2", bufs=4, name="tc2")
                    nc.scalar.activation(tc_[:], c_f[:, i, :], AF.Tanh)
                    nc.vector.tensor_tensor(
                        out=h_b[:, i, :], in0=gates[6 + i][:, :1], in1=tc_[:],
                        op=AluOpType.mult,
                    )
                return h_b, c_f

            f2h0b, f2c0 = l2_step(0, 0, None, None)
            f2h1b, _ = l2_step(0, 1, f2h0b, f2c0)
            b2h0b, b2c0 = l2_step(1, 1, None, None)
            b2h1b, _ = l2_step(1, 0, b2h0b, b2c0)

            # flatT [128, 4, 2]: t0 = [f2h0, b2h1], t1 = [f2h1, b2h0]
            flatT = sb.tile([128, 4, 2], BF16, tag="flatT", bufs=1, name="flatT")
            for t_, (fh, bh) in enumerate([(f2h0b, b2h1b), (f2h1b, b2h0b)]):
                for i in range(2):
                    nc.vector.tensor_copy(flatT[:, i, t_ : t_ + 1], fh[:, i, :])
                    nc.vector.tensor_copy(flatT[:, 2 + i, t_ : t_ + 1], bh[:, i, :])

            wtag_sb = sb.tile([128, 4, 1], BF16, tag="wtag", bufs=1, name="wtag_sb")
            nc.sync.dma_start(
                out=wtag_sb[:], in_=wtag.ap().rearrange("(k p) m -> p k m", p=128)
            )
            btag_sb = sb.tile([1, 1], F32, tag="btag", bufs=1, name="btag_sb")
            nc.sync.dma_start(out=btag_sb[:], in_=btag.ap())
            fo = ps_sps.tile([1, 2], F32, tag="sps", name="fo")
            for kt in range(4):
                nc.tensor.matmul(
                    fo[:], lhsT=wtag_sb[:, kt, :], rhs=flatT[:, kt, :],
                    start=(kt == 0), stop=(kt == 3),
                )
            out_sb = sb.tile([1, 2], F32, tag="outsb", bufs=1, name="out_sb")
            nc.scalar.activation(out_sb[:], fo[:], AF.Identity, bias=btag_sb[:1, :1])
            nc.sync.dma_start(out=out.ap(), in_=out_sb[:])

    nc.finalize()
    return nc


def prep_inputs(inputs, L=LMAX):
    """Host-side sharding/layout prep. Returns in_maps for 8 cores."""
    f32 = lambda a: np.ascontiguousarray(np.asarray(a), dtype=np.float32)
    bf = lambda a: np.ascontiguousarray(np.asarray(np.asarray(a), dtype=np.float32).astype(BF))

    sentences = np.asarray(inputs["sentences"], dtype=np.int32)
    shared = {
        "word_emb": bf(inputs["word_emb"]),
        "embs": f32(inputs["emb_ln_s"]).reshape(1, D),
        "embb": f32(inputs["emb_ln_b"]).reshape(1, D),
        "wq": bf(inputs["Wq"][:L]), "wk": bf(inputs["Wk"][:L]),
        "wv": bf(inputs["Wv"][:L]), "wo": bf(inputs["Wo"][:L]),
        "bq": f32(inputs["bq"][:L]), "bk": f32(inputs["bk"][:L]),
        "bo": f32(
            np.asarray(inputs["bo"][:L], dtype=np.float32)
            + np.einsum(
                "ld,ldo->lo",
                np.asarray(inputs["bv"][:L], dtype=np.float32),
                np.asarray(inputs["Wo"][:L], dtype=np.float32),
            )
        ),
        "ln1s": f32(inputs["ln1_s"][:L]), "ln1b": f32(inputs["ln1_b"][:L]),
        "ln2s": f32(inputs["ln2_s"][:L]), "ln2b": f32(inputs["ln2_b"][:L]),
        "wf1": bf(inputs["Wf1"][:L]), "bf1": f32(inputs["bf1"][:L]),
        "wf2": bf(inputs["Wf2"][:L]), "bf2": f32(inputs["bf2"][:L]),
        "l1wih": bf(np.transpose(np.asarray(inputs["l1_Wih"]), (0, 2, 1))),
        "l1whh": bf(np.transpose(np.asarray(inputs["l1_Whh"]), (0, 2, 1))),
        "l1bih": f32(inputs["l1_bih"]), "l1bhh": f32(inputs["l1_bhh"]),
        "paw1": bf(inputs["pa_W1"]), "pab1": f32(inputs["pa_b1"]).reshape(64, 1),
        "paw2": bf(inputs["pa_W2"]), "pab2": f32(inputs["pa_b2"]).reshape(1, 1),
        "l2wih": bf(np.transpose(np.asarray(inputs["l2_Wih"]), (0, 2, 1))),
        "l2whh": bf(np.transpose(np.asarray(inputs["l2_Whh"]), (0, 2, 1))),
        "l2bih": f32(inputs["l2_bih"]), "l2bhh": f32(inputs["l2_bhh"]),
        "wtag": bf(inputs["Wtag"]), "btag": f32(inputs["btag"]).reshape(1, 1),
    }
    pos_emb = f32(inputs["pos_emb"])

    in_maps = []
    for g in range(NC):
        b, q = g // 4, g % 4
        pos0 = 512 * q - 256
        rows = np.arange(R) + pos0
        rows_c = np.clip(rows, 0, S - 1)
        m = dict(shared)
        m["sent_idx"] = np.ascontiguousarray(
            sentences[b][rows_c].reshape(R, 1).astype(np.int32)
        )
        m["pos_sl"] = np.ascontiguousarray(pos_emb[rows_c])
        # masks [2 chunks, 4 slots, 128 keys(p), 256 queries(i)] — 0/1 validity
        mk = np.zeros((2, 4, 128, 256), dtype=np.float32)
        slot_wk = [0, 1, 4, 5]
        i_idx = np.arange(256)
        p_idx = np.arange(128)
        for c in range(2):
            qpos = 512 * q + 256 * c + i_idx  # [256] global query positions
            for s_, wk_ in enumerate(slot_wk):
                kpos = pos0 + 256 * c + wk_ * 128 + p_idx  # [128] global key pos
                valid = (
                    (np.abs(kpos[:, None] - qpos[None, :]) <= W)
                    & (kpos[:, None] >= 0)
                    & (kpos[:, None] < S)
                )
                mk[c, s_][valid] = 1.0
        m["masks"] = np.ascontiguousarray(mk.reshape(2 * 4 * 128, 256).astype(BF))
        # picks [4, KT, 128]: left-halo rank, right-halo rank, batch0 rank, batch1 rank
        d = np.arange(D)
        gl = g - 1 if q > 0 else g
        gr = g + 1 if q < 3 else g
        pk = np.stack(
            [gl * D + d, gr * D + d, q * D + d, (q + 4) * D + d]
        ).reshape(4, KT, 128)
        m["picks"] = np.ascontiguousarray(pk.reshape(4 * KT * 128, 1).astype(np.int32))
        in_maps.append(m)
    return in_maps


_NC_CACHE = {}


def run(inputs, L=LMAX, trace=False):
    if L not in _NC_CACHE:
        _NC_CACHE[L] = build_nc(L)
    nc = _NC_CACHE[L]
    in_maps = prep_inputs(inputs, L)
    res = run_bass_kernel_spmd(nc, in_maps, core_ids=list(range(NC)), trace=trace)
    out = np.asarray(res.results[0]["out"], dtype=np.float32).reshape(B, 1)
    return out, res


def kernel(**inputs) -> np.ndarray:
    out, _ = run(inputs, L=LMAX, trace=False)
    return out



# revision 45
# speedup vs baseline: 1.1733x; 1.0705x over previous
"""Distributed Trainium2 kernel for nn_ADUPredictor (12-layer sliding-window
encoder + BiLSTM/attention-pool tail), SPMD across 8 NeuronCores.

Sharding: cores 0-3 = batch 0, cores 4-7 = batch 1; core g owns the 512-token
sequence quarter q = g%4 plus 256-token halos on each side. Activations are
kept feature-major ("T layout", [feature, token]) so every projection uses the
weight as the stationary matmul operand. Per-layer halo exchange = AllGather of
each core's own x (bf16) + indirect-DMA picks of the two neighbor boundary
blocks. The BiLSTM/pool tail runs per seq-quarter (both batches) with one small
AllReduce for the global softmax pooling; LSTM2 + tag head are replicated.
"""

import sys

if "/opt/trn_rl_repo" not in sys.path:
    sys.path.insert(0, "/opt/trn_rl_repo")

import numpy as np
import ml_dtypes

import concourse.bass as bass
import concourse.mybir as mybir
import concourse.tile as tile
from concourse import bacc
from concourse.bass import ds, ts
from concourse.bass_utils import run_bass_kernel_spmd
from concourse.masks import make_identity
from concourse.alu_op_type import AluOpType
import bass_rust

F32 = mybir.dt.float32
F8 = mybir.dt.float8e4
DR = mybir.MatmulPerfMode.DoubleRow
W2SCALE = 16.0
BF16 = mybir.dt.float16
I32 = mybir.dt.int32
AF = mybir.ActivationFunctionType
AX = bass_rust.AxisListType

B, S = 2, 2048
D, H, DH, W, LMAX, FF = 768, 12, 64, 256, 12, 3072
VOCAB = 50265
HID, HALF = 512, 256
KT = D // 128          # 6 feature tiles
RT = 8                 # 1024 rows / 128
OWN = 512              # own rows per core
R = 1024               # own + halos
NC = 8
EPS = 1e-5
MASKVAL = -50.0

BF = np.float16


def build_nc(L=LMAX):
    nc = bacc.Bacc(None)

    def param(name, shape, dt):
        return nc.declare_dram_parameter(name, list(shape), dt, isOutput=False)

    sent_idx = param("sent_idx", [R, 1], I32)
    pos_sl = param("pos_sl", [R, D], F32)
    masks = param("masks", [2 * 4 * 128, 256], BF16)
    picks = param("picks", [4 * KT * 128, 1], I32)

    word_emb = param("word_emb", [VOCAB, D], BF16)
    embs = param("embs", [1, D], F32)
    embb = param("embb", [1, D], F32)
    wq = param("wq", [L, D, D], BF16)
    wk = param("wk", [L, D, D], BF16)
    wv = param("wv", [L, D, D], BF16)
    wo = param("wo", [L, D, D], BF16)
    bq = param("bq", [L, D], F32)
    bk = param("bk", [L, D], F32)
    bo = param("bo", [L, D], F32)
    ln1s = param("ln1s", [L, D], F32)
    ln1b = param("ln1b", [L, D], F32)
    ln2s = param("ln2s", [L, D], F32)
    ln2b = param("ln2b", [L, D], F32)
    wf1 = param("wf1", [L, D, FF], BF16)
    bf1 = param("bf1", [L, FF], F32)
    wf2 = param("wf2", [L, FF, D], BF16)
    bf2 = param("bf2", [L, D], F32)

    l1wih = param("l1wih", [2, D, 4 * HALF], BF16)     # transposed [in, 4*HALF]
    l1whh = param("l1whh", [2, HALF, 4 * HALF], BF16)
    l1bih = param("l1bih", [2, 4 * HALF], F32)
    l1bhh = param("l1bhh", [2, 4 * HALF], F32)
    paw1 = param("paw1", [HID, 64], BF16)
    pab1 = param("pab1", [64, 1], F32)
    paw2 = param("paw2", [64, 1], BF16)
    pab2 = param("pab2", [1, 1], F32)
    l2wih = param("l2wih", [2, HID, 4 * HALF], BF16)
    l2whh = param("l2whh", [2, HALF, 4 * HALF], BF16)
    l2bih = param("l2bih", [2, 4 * HALF], F32)
    l2bhh = param("l2bhh", [2, 4 * HALF], F32)
    wtag = param("wtag", [HID, 1], BF16)
    btag = param("btag", [1, 1], F32)

    out = nc.declare_dram_parameter("out", [1, 2], F32, isOutput=True)

    with tile.TileContext(nc) as tc:
        with (
            tc.tile_pool(name="pers", bufs=1) as pers,
            tc.tile_pool(name="sb", bufs=1) as sb,
            tc.tile_pool(name="ps_sps", bufs=2, space="PSUM") as ps_sps,
            tc.tile_pool(name="ps_ctx", bufs=2, space="PSUM") as ps_ctx,
            tc.tile_pool(name="ps_big", bufs=2, space="PSUM") as ps_big,
            tc.tile_pool(name="dram", bufs=2, space="DRAM") as dram,
        ):
            # ---------------- persistent constants ----------------
            ones_col = pers.tile([128, 1], BF16)   # lhsT for partition sums
            nc.vector.memset(ones_col[:], 1.0)
            ones_row_f = pers.tile([1, 128], F32)  # lhsT for PE row-broadcasts
            nc.vector.memset(ones_row_f[:], 1.0)
            ones_row_bf = pers.tile([1, 128], BF16)
            nc.vector.memset(ones_row_bf[:], 1.0)
            ident = pers.tile([128, 128], BF16)
            make_identity(nc, ident[:])

            v_buf = pers.tile([128, RT, H * 65], BF16)
            nc.vector.memset(
                v_buf[:].rearrange("p r (h e) -> p (r h) e", e=65)[:, :, 64:65], 1.0
            )

            masks_sb = pers.tile([128, 8, 256], BF16)
            nc.sync.dma_start(
                out=masks_sb[:],
                in_=masks.ap().rearrange("(m p) q -> p m q", p=128),
            )

            idx_sb = pers.tile([128, RT, 1], I32)
            nc.sync.dma_start(
                out=idx_sb[:], in_=sent_idx.ap().rearrange("(t p) o -> p t o", p=128)
            )
            eps_sb = pers.tile([128, 1], F32)
            nc.vector.memset(eps_sb[:], EPS)
            embs_row = pers.tile([1, D], F32)
            nc.sync.dma_start(out=embs_row[:], in_=embs.ap())
            embb_row = pers.tile([1, D], F32)
            nc.sync.dma_start(out=embb_row[:], in_=embb.ap())
            embs_row_bf = pers.tile([1, D], BF16)
            nc.vector.tensor_copy(embs_row_bf[:], embs_row[:])
            embb_row_bf = pers.tile([1, D], BF16)
            nc.vector.tensor_copy(embb_row_bf[:], embb_row[:])

            # scratch tag helpers (sizes chosen as max over all users)
            def t_exp(name):   # 2KB slot: exp tiles, lstm gates, emb gathers
                return sb.tile([128, 512], F32, tag="exp", bufs=10, name=name)

            def t_exp_bf(name):
                return sb.tile([128, 512], BF16, tag="exp", bufs=10, name=name)

            # ---------------- embedding (per row-tile, streaming) ----------------
            x_bf = sb.tile([128, KT, R], BF16, tag="xbf", bufs=1, name="x_bf0")
            x_f = sb.tile([128, KT, OWN], BF16, tag="xf", bufs=1, name="x_f0")
            embs_full = sb.tile([128, D], BF16, tag="lnt", bufs=4, name="embs_full")
            nc.gpsimd.partition_broadcast(embs_full[:], embs_row_bf[:])
            embb_full = sb.tile([128, D], BF16, tag="lnt", bufs=4, name="embb_full")
            nc.gpsimd.partition_broadcast(embb_full[:], embb_row_bf[:])
            for rt in range(RT):
                xg = sb.tile([128, D], BF16, tag="exp", bufs=10, name="xg")
                nc.gpsimd.indirect_dma_start(
                    out=xg[:],
                    out_offset=None,
                    in_=word_emb.ap(),
                    in_offset=bass.IndirectOffsetOnAxis(ap=idx_sb[:, rt, :], axis=0),
                )
                pos_rt = sb.tile([128, D], F32, tag="wqkvo", bufs=2, name="pos_rt")
                nc.sync.dma_start(out=pos_rt[:], in_=pos_sl.ap()[ts(rt, 128), :])
                xe = sb.tile([128, D], F32, tag="wf1t", bufs=2, name="xe")
                nc.vector.tensor_tensor(
                    out=xe[:], in0=xg[:], in1=pos_rt[:], op=AluOpType.add
                )
                # row-wise LN over free axis (features)
                m = sb.tile([128, 1], F32, tag="lnrow", bufs=8, name="m")
                nc.vector.reduce_sum(m[:], xe[:], axis=AX.X)
                nc.scalar.mul(m[:], m[:], 1.0 / D)
                xc = sb.tile([128, D], F32, tag="wf1t", bufs=2, name="xc")
                nc.vector.tensor_scalar_sub(xc[:], xe[:], m[:, :1])
                sq = sb.tile([128, D], F32, tag="wqkvo", bufs=2, name="sq")
                nc.scalar.activation(sq[:], xc[:], AF.Square)
                v_ = sb.tile([128, 1], F32, tag="lnrow", bufs=8, name="v_")
                nc.vector.reduce_sum(v_[:], sq[:], axis=AX.X)
                lnv = sb.tile([128, 1], F32, tag="lnrow", bufs=8, name="lnv")
                nc.scalar.activation(lnv[:], v_[:], AF.Ln, bias=eps_sb[:, :1], scale=1.0 / D)
                rstd = sb.tile([128, 1], F32, tag="lnrow", bufs=8, name="rstd")
                nc.scalar.activation(rstd[:], lnv[:], AF.Exp, scale=-0.5)
                nc.vector.tensor_scalar_mul(xc[:], xc[:], rstd[:, :1])
                nc.vector.tensor_tensor(out=xc[:], in0=xc[:], in1=embs_full[:], op=AluOpType.mult)
                xn_bf = sb.tile([128, D], BF16, tag="exp", bufs=10, name="xn_bf")
                nc.vector.tensor_tensor(
                    out=xn_bf[:], in0=xc[:], in1=embb_full[:], op=AluOpType.add
                )
                for kt in range(KT):
                    tp = ps_sps.tile([128, 128], BF16, tag="sps", name="tp")
                    nc.tensor.transpose(tp[:], xn_bf[:, ts(kt, 128)], ident[:])
                    nc.scalar.activation(x_bf[:, kt, ts(rt, 128)], tp[:], AF.Copy)
                    if 2 <= rt <= 5:
                        nc.vector.tensor_copy(x_f[:, kt, ts(rt - 2, 128)], tp[:])

            # ---------------- helpers ----------------
            def load_wmat(p, l, name):
                t = sb.tile([128, KT, D], BF16, tag="wqkvo", bufs=2, name=name)
                nc.sync.dma_start(
                    out=t[:], in_=p.ap()[l].rearrange("(k p) m -> p k m", p=128)
                )
                return t

            def load_vec(p, l, name, scale=None):
                t = sb.tile([128, KT], F32, tag="pvec", bufs=14, name=name)
                nc.sync.dma_start(
                    out=t[:], in_=p.ap()[l].rearrange("(k p) -> p k", p=128)
                )
                if scale is not None:
                    nc.scalar.mul(t[:], t[:], scale)
                return t

            def ln_T(xr, s_sb, b_sb, out_f, out_bf, out_bf_off):
                """LayerNorm over the partition (feature) axis of xr [128,KT,OWN].

                If out_f is not None writes f32 result there and a bf16 copy to
                out_bf; else writes bf16 result directly to out_bf."""
                xr_bf = sb.tile([128, KT, OWN], BF16, tag="xrbf", bufs=1, name="xr_bf")
                xsq_bf = sb.tile([128, KT, OWN], BF16, tag="xsq", bufs=1, name="xsq_bf")
                for kt in range(KT):
                    nc.vector.tensor_copy(xr_bf[:, kt, :], xr[:, kt, :])
                    nc.scalar.activation(xsq_bf[:, kt, :], xr[:, kt, :], AF.Square)
                sum_ps = ps_sps.tile([1, OWN], F32, tag="sps", name="sum_ps")
                sq_ps = ps_sps.tile([1, OWN], F32, tag="sps", name="sq_ps")
                for kt in range(KT):
                    nc.tensor.matmul(
                        sum_ps[:], lhsT=ones_col[:], rhs=xr_bf[:, kt, :],
                        start=(kt == 0), stop=(kt == KT - 1),
                    )
                for kt in range(KT):
                    nc.tensor.matmul(
                        sq_ps[:], lhsT=ones_col[:], rhs=xsq_bf[:, kt, :],
                        start=(kt == 0), stop=(kt == KT - 1),
                    )
                m = sb.tile([1, OWN], F32, tag="lncol", bufs=6, name="m")
                nc.scalar.activation(m[:], sum_ps[:], AF.Copy, scale=1.0 / D)
                msq = sb.tile([1, OWN], F32, tag="lncol", bufs=6, name="msq")
                nc.vector.tensor_tensor(out=msq[:], in0=m[:], in1=m[:], op=AluOpType.mult)
                var = sb.tile([1, OWN], F32, tag="lncol", bufs=6, name="var")
                nc.vector.scalar_tensor_tensor(
                    out=var[:], in0=sq_ps[:], scalar=1.0 / D, in1=msq[:],
                    op0=AluOpType.mult, op1=AluOpType.subtract,
                )
                lnv = sb.tile([1, OWN], F32, tag="lncol", bufs=6, name="lnv")
                nc.scalar.activation(lnv[:], var[:], AF.Ln, bias=eps_sb[:1, :1])
                # bf16 rows -> PE rank-1 broadcast -> bf16 SBUF copies; the whole
                # apply then runs at DVE 2x/4x bf16 rates
                rstd_rb = sb.tile([1, OWN], BF16, tag="lncol", bufs=6, name="rstd_rb")
                nc.scalar.activation(rstd_rb[:], lnv[:], AF.Exp, scale=-0.5)
                mrs_rb = sb.tile([1, OWN], BF16, tag="lncol", bufs=6, name="mrs_rb")
                nc.vector.tensor_tensor(out=mrs_rb[:], in0=m[:], in1=rstd_rb[:], op=AluOpType.mult)
                rstd_ps = ps_ctx.tile([128, OWN], F32, tag="ctx", name="rstd_ps")
                nc.tensor.matmul(
                    rstd_ps[:], lhsT=ones_row_bf[:1, :], rhs=rstd_rb[:1, :],
                    start=True, stop=True,
                )
                mrs_ps = ps_ctx.tile([128, OWN], F32, tag="ctx", name="mrs_ps")
                nc.tensor.matmul(
                    mrs_ps[:], lhsT=ones_row_bf[:1, :], rhs=mrs_rb[:1, :],
                    start=True, stop=True,
                )
                rstd_fb = sb.tile([128, OWN], BF16, tag="lnt", bufs=4, name="rstd_fb")
                nc.scalar.activation(rstd_fb[:], rstd_ps[:], AF.Copy)
                mrs_fb = sb.tile([128, OWN], BF16, tag="lnt", bufs=4, name="mrs_fb")
                nc.scalar.activation(mrs_fb[:], mrs_ps[:], AF.Copy)
                for kt in range(KT):
                    t1 = sb.tile([128, OWN], BF16, tag="lnap", bufs=3, name="t1")
                    nc.vector.tensor_tensor(
                        out=t1[:], in0=xr_bf[:, kt, :], in1=rstd_fb[:], op=AluOpType.mult
                    )
                    nc.vector.tensor_tensor(
                        out=t1[:], in0=t1[:], in1=mrs_fb[:], op=AluOpType.subtract
                    )
                    nc.vector.tensor_scalar(
                        out=out_bf[:, kt, ds(out_bf_off, OWN)], in0=t1[:],
                        scalar1=s_sb[:, kt : kt + 1], scalar2=b_sb[:, kt : kt + 1],
                        op0=AluOpType.mult, op1=AluOpType.add,
                    )
                    if out_f is not None:
                        nc.vector.tensor_copy(
                            out_f[:, kt, :], out_bf[:, kt, ds(out_bf_off, OWN)]
                        )

            # masked rt map: rt -> (masked chunk, mask slot)
            mask_slot = {0: (0, 0), 1: (0, 1), 2: (1, 0), 3: (1, 1),
                         4: (0, 2), 5: (0, 3), 6: (1, 2), 7: (1, 3)}

            # ---- per-core halo source rows, computed in SP registers:
            # q = pid%4; gl = pid-1+((4-q)//4); gr = pid+1-((q+1)//4); offsets *D
            pid = nc.sync.partition_id()
            q_r = nc.sync.alloc_register("q_r")
            nc.sync.reg_mod(q_r, pid, 4)
            t0_r = nc.sync.alloc_register("t0_r")
            nc.sync.reg_sub(t0_r, 4, q_r)
            nc.sync.reg_div(t0_r, t0_r, 4)          # 1 iff q==0
            gl_r = nc.sync.alloc_register("gl_r")
            nc.sync.reg_add(gl_r, pid, t0_r)
            nc.sync.reg_sub(gl_r, gl_r, 1)
            nc.sync.reg_mul(gl_r, gl_r, D)
            t1_r = nc.sync.alloc_register("t1_r")
            nc.sync.reg_add(t1_r, q_r, 1)
            nc.sync.reg_div(t1_r, t1_r, 4)          # 1 iff q==3
            gr_r = nc.sync.alloc_register("gr_r")
            nc.sync.reg_sub(gr_r, pid, t1_r)
            nc.sync.reg_add(gr_r, gr_r, 1)
            nc.sync.reg_mul(gr_r, gr_r, D)
            glD_sv = nc.sync.snap(gl_r, min_val=0, max_val=(NC - 1) * D)
            grD_sv = nc.sync.snap(gr_r, min_val=0, max_val=(NC - 1) * D)
            # tail gathers: rows q*D (batch 0) and (q+4)*D (batch 1)
            qD_r = nc.sync.alloc_register("qD_r")
            nc.sync.reg_mul(qD_r, q_r, D)
            q4D_r = nc.sync.alloc_register("q4D_r")
            nc.sync.reg_add(q4D_r, qD_r, 4 * D)
            qD_sv = nc.sync.snap(qD_r, min_val=0, max_val=3 * D)
            q4D_sv = nc.sync.snap(q4D_r, min_val=4 * D, max_val=7 * D)

            x_bf_cur, x_f_cur = x_bf, x_f

            for l in range(L):
                wq_sb = load_wmat(wq, l, "wq_sb")
                wk_sb = load_wmat(wk, l, "wk_sb")
                bq8 = load_vec(bq, l, "bq8", scale=0.125)
                bk_sb = load_vec(bk, l, "bk_sb")
                bo_sb = load_vec(bo, l, "bo_sb")
                ln1s_sb = load_vec(ln1s, l, "ln1s_sb")
                ln1b_sb = load_vec(ln1b, l, "ln1b_sb")
                ln2s_sb = load_vec(ln2s, l, "ln2s_sb")
                ln2b_sb = load_vec(ln2b, l, "ln2b_sb")
                bf2_sb = load_vec(bf2, l, "bf2_sb")
                bf1_sb = sb.tile([128, FF // 128], F32, tag="bf1", bufs=2, name="bf1_sb")
                nc.sync.dma_start(
                    out=bf1_sb[:], in_=bf1.ap()[l].rearrange("(k p) -> p k", p=128)
                )

                # ---- halo exchange for this layer's x (l==0 computed locally)
                if l > 0:
                  with tc.high_priority():
                      send = dram.tile([D, OWN], BF16, tag="send", name="send")
                      nc.sync.dma_start(
                          out=send.rearrange("(k p) c -> p k c", p=128),
                          in_=x_bf_cur[:, :, 256 : 256 + OWN],
                      )
                      gathered = dram.tile(
                          [NC * D, OWN], BF16, addr_space="Shared", tag="gath", name="gathered"
                      )
                      nc.gpsimd.collective_compute(
                          "AllGather",
                          AluOpType.bypass,
                          ins=[send.opt()],
                          outs=[gathered.opt()],
                          replica_groups=[list(range(NC))],
                      )
                      nc.sync.dma_start(
                          out=x_bf_cur[:, :, 0:256],
                          in_=gathered[ds(glD_sv, D), ds(256, 256)].rearrange(
                              "(k p) c -> p k c", p=128
                          ),
                      )
                      nc.sync.dma_start(
                          out=x_bf_cur[:, :, 768:1024],
                          in_=gathered[ds(grD_sv, D), ds(0, 256)].rearrange(
                              "(k p) c -> p k c", p=128
                          ),
                      )

                # ---- qT (own rows), kT own
                qT_sb = sb.tile([128, KT, OWN], BF16, tag="qT", bufs=1, name="qT_sb")
                kT_sb = sb.tile([128, KT, R], BF16, tag="kT", bufs=1, name="kT_sb")
                for mt in range(KT):
                    qp = ps_big.tile([128, OWN], F32, tag="big", name="qp")
                    for kt in range(KT):
                        nc.tensor.matmul(
                            qp[:], lhsT=wq_sb[:, kt, ts(mt, 128)],
                            rhs=x_bf_cur[:, kt, 256 : 256 + OWN],
                            start=(kt == 0), stop=(kt == KT - 1),
                        )
                    nc.scalar.activation(
                        qT_sb[:, mt, :], qp[:], AF.Identity,
                        bias=bq8[:, mt : mt + 1], scale=0.125,
                    )
                    kp = ps_big.tile([128, OWN], F32, tag="big", name="kp")
                    for kt in range(KT):
                        nc.tensor.matmul(
                            kp[:], lhsT=wk_sb[:, kt, ts(mt, 128)],
                            rhs=x_bf_cur[:, kt, 256 : 256 + OWN],
                            start=(kt == 0), stop=(kt == KT - 1),
                        )
                    nc.scalar.activation(
                        kT_sb[:, mt, 256 : 256 + OWN], kp[:], AF.Identity,
                        bias=bk_sb[:, mt : mt + 1],
                    )

                wv_sb = load_wmat(wv, l, "wv_sb")

                def v_rows(rt):
                    for nh in range(2):
                        vp = ps_big.tile([128, 384], F32, tag="big", name="vp")
                        for kt in range(KT):
                            nc.tensor.matmul(
                                vp[:], lhsT=x_bf_cur[:, kt, ts(rt, 128)],
                                rhs=wv_sb[:, kt, ts(nh, 384)],
                                start=(kt == 0), stop=(kt == KT - 1),
                            )
                        nc.scalar.activation(
                            v_buf[:, rt, :].rearrange("p (h e) -> p h e", e=65)[
                                :, ds(nh * 6, 6), 0:64
                            ],
                            vp[:].rearrange("p (h e) -> p h e", e=64),
                            AF.Copy,
                        )

                for rt in (2, 3, 4, 5):
                    v_rows(rt)

                # ---- halo kT, v (depend on picks)
                for mt in range(KT):
                    for side in range(2):
                        hoff = 0 if side == 0 else 768
                        kp2 = ps_big.tile([128, 256], F32, tag="big", name="kp2")
                        for kt in range(KT):
                            nc.tensor.matmul(
                                kp2[:], lhsT=wk_sb[:, kt, ts(mt, 128)],
                                rhs=x_bf_cur[:, kt, ds(hoff, 256)],
                                start=(kt == 0), stop=(kt == KT - 1),
                            )
                        nc.scalar.activation(
                            kT_sb[:, mt, ds(hoff if side == 0 else 768, 256)], kp2[:],
                            AF.Identity, bias=bk_sb[:, mt : mt + 1],
                        )
                for rt in (0, 1, 6, 7):
                    v_rows(rt)

                # ---- attention per head: scores+exp (8 rts) then ctx (2 chunks)
                ctxT_sb = sb.tile([128, KT, OWN], BF16, tag="ctxT", bufs=1, name="ctxT_sb")
                # unnormalized ctx stash (2 heads packed per partition group, slots
                # reused per 6-head half) + packed softmax denominators; reciprocal
                # runs batched over 12 (head, chunk) rows at a time
                ctxu_all = sb.tile([128, 3, 2, 256], F32, tag="ctxu", bufs=1, name="ctxu_all")
                # halves live at partition bases 0 and 32 (engine bases are 32-aligned)
                den_all = sb.tile([44, 256], F32, tag="den", bufs=2, name="den_all")
                rec_all = sb.tile([44, 256], F32, tag="den", bufs=2, name="rec_all")

                def den_slot(h, c):
                    return (0 if h < 6 else 32) + 2 * (h % 6) + c

                def ctx_norm_flush(h0):
                    base = 0 if h0 == 0 else 32
                    nc.vector.reciprocal(
                        rec_all[ds(base, 12), :], den_all[ds(base, 12), :]
                    )
                    # bounce via DRAM: partition-stride-0 reads are DRAM-only
                    rec_dr = dram.tile([12, 256], F32, tag="recdr", name="rec_dr")
                    nc.sync.dma_start(out=rec_dr[:], in_=rec_all[ds(base, 12), :])
                    for h_ in range(h0, h0 + 6):
                        kq_, po_ = h_ // 2, (h_ % 2) * 64
                        for c_ in range(2):
                            sl_ = 2 * (h_ - h0) + c_
                            recf = sb.tile([128, 256], F32, tag="recf", bufs=4, name="recf")
                            nc.sync.dma_start(
                                out=recf[ds(po_, 64), :],
                                in_=rec_dr[sl_ : sl_ + 1, :].to_broadcast((64, 256)),
                            )
                            nc.vector.tensor_tensor(
                                out=ctxT_sb[ds(po_, 64), kq_, ds(c_ * 256, 256)],
                                in0=ctxu_all[ds(po_, 64), kq_ % 3, c_, :],
                                in1=recf[ds(po_, 64), :],
                                op=AluOpType.mult,
                            )

                for kq in range(KT):
                    ets = {}
                    for rt in (2, 3, 4, 5, 0, 1, 6, 7):
                        c_m, slot = mask_slot[rt]
                        if rt in (0, 1):
                            qoff, nq = 0, 256
                        elif rt in (6, 7):
                            qoff, nq = 256, 256
                        else:
                            qoff, nq = 0, 512
                        moff = c_m * 256
                        # adjacent row-tiled MMs (bases 0 and 64) run concurrently
                        # on disjoint halves of the PE array
                        spp = ps_sps.tile([128, 2, 512], F32, tag="sps", name="spp")
                        for i in range(2):
                            nc.tensor.matmul(
                                spp[:, i, ds(qoff, nq)],
                                lhsT=kT_sb[ds(i * 64, 64), kq, ts(rt, 128)],
                                rhs=qT_sb[ds(i * 64, 64), kq, ds(qoff, nq)],
                                start=True, stop=True,
                            )
                        etp = sb.tile([128, 2, 512], BF16, tag="exp", bufs=10, name="etp")
                        # 256-col-per-bank ACT ops over both heads at once
                        for eo in range(qoff, qoff + nq, 256):
                            nc.scalar.activation(
                                etp[:, :, ds(eo, 256)], spp[:, :, ds(eo, 256)], AF.Exp
                            )
                        # zero invalid band positions (masks are 0/1 bf16)
                        for i in range(2):
                            nc.vector.tensor_tensor(
                                out=etp[:, i, ds(moff, 256)], in0=etp[:, i, ds(moff, 256)],
                                in1=masks_sb[:, c_m * 4 + slot, :], op=AluOpType.mult,
                            )
                        ets[rt] = etp
                    for i in range(2):
                        h, po = 2 * kq + i, i * 64
                        for c in range(2):
                            rts = (2, 3, 4, 5, 0, 1) if c == 0 else (2, 3, 4, 5, 6, 7)
                            cp = ps_ctx.tile([65, 256], F32, tag="ctx", name="cp")
                            for j, rt in enumerate(rts):
                                nc.tensor.matmul(
                                    cp[:],
                                    lhsT=v_buf[:, rt, ds(h * 65, 65)],
                                    rhs=ets[rt][:, i, ds(c * 256, 256)],
                                    start=(j == 0), stop=(j == 5),
                                )
                            nc.vector.tensor_copy(
                                ctxu_all[ds(po, 64), kq % 3, c, :], cp[0:64, :]
                            )
                            # den row -> SBUF (lane 64; engine partition bases must
                            # be 32-aligned), then DMA into the packed per-slot lane
                            den_sb = sb.tile([65, 256], F32, tag="densb", bufs=4, name="den_sb")
                            nc.vector.tensor_copy(den_sb[64:65, :], cp[64:65, :])
                            sl = den_slot(h, c)
                            nc.sync.dma_start(
                                out=den_all[sl : sl + 1, :], in_=den_sb[64:65, :]
                            )
                    if kq == 2:
                        ctx_norm_flush(0)
                ctx_norm_flush(6)

                # ---- Wo + residual + LN1
                wo_sb = load_wmat(wo, l, "wo_sb")
                xr = sb.tile([128, KT, OWN], F32, tag="xr", bufs=1, name="xr")
                for mt in range(KT):
                    op_ = ps_big.tile([128, OWN], F32, tag="big", name="op_")
                    for kt in range(KT):
                        nc.tensor.matmul(
                            op_[:], lhsT=wo_sb[:, kt, ts(mt, 128)],
                            rhs=ctxT_sb[:, kt, :],
                            start=(kt == 0), stop=(kt == KT - 1),
                        )
                    nc.vector.scalar_tensor_tensor(
                        out=xr[:, mt, :], in0=op_[:], scalar=bo_sb[:, mt : mt + 1],
                        in1=x_f_cur[:, mt, :], op0=AluOpType.add, op1=AluOpType.add,
                    )
                xp_bf = sb.tile([128, KT, OWN], BF16, tag="qT", bufs=1, name="xp_bf")
                ln_T(xr, ln1s_sb, ln1b_sb, None, xp_bf, 0)

                # ---- FFN1 (+gelu) -> h_bf
                h_lo = sb.tile([128, 12, OWN], BF16, tag="kT", bufs=1, name="h_lo")
                h_hi = sb.tile([128, 12, OWN], BF16, tag="hbuf", bufs=1, name="h_hi")

                def h_sl(mt):
                    return h_lo[:, mt, :] if mt < 12 else h_hi[:, mt - 12, :]
                for mtb in range(8):
                    w1t = sb.tile([128, KT, 384], BF16, tag="wf1t", bufs=2, name="w1t")
                    nc.sync.dma_start(
                        out=w1t[:],
                        in_=wf1.ap()[l].rearrange("(k p) m -> p k m", p=128)[
                            :, :, ds(mtb * 384, 384)
                        ],
                    )
                    for mi in range(3):
                        mt = mtb * 3 + mi
                        fp = ps_big.tile([128, OWN], F32, tag="big", name="fp")
                        for kt in range(KT):
                            nc.tensor.matmul(
                                fp[:], lhsT=w1t[:, kt, ts(mi, 128)],
                                rhs=xp_bf[:, kt, :],
                                start=(kt == 0), stop=(kt == KT - 1),
                            )
                        nc.scalar.activation(
                            h_sl(mt), fp[:], AF.Gelu_apprx_tanh,
                            bias=bf1_sb[:, mt : mt + 1],
                        )

                # ---- FFN2 + residual + LN2 -> next x
                xr2 = sb.tile([128, KT, OWN], F32, tag="xr", bufs=1, name="xr2")
                for mp in range(3):  # pairs of output tiles
                    f2ps = [
                        ps_big.tile([128, OWN], F32, tag="big", name=f"f2p{j}")
                        for j in range(2)
                    ]
                    for ktb in range(4):
                        w2t = sb.tile([128, KT, 256], BF16, tag="wf2t", bufs=2, name="w2t")
                        nc.sync.dma_start(
                            out=w2t[:],
                            in_=wf2.ap()[l].rearrange("(k p) m -> p k m", p=128)[
                                :, ds(ktb * KT, KT), ds(mp * 256, 256)
                            ],
                        )
                        for kt in range(KT):
                            for j in range(2):
                                nc.tensor.matmul(
                                    f2ps[j][:],
                                    lhsT=w2t[:, kt, ts(j, 128)],
                                    rhs=h_sl(ktb * KT + kt),
                                    start=(ktb == 0 and kt == 0),
                                    stop=(ktb == 3 and kt == KT - 1),
                                )
                    for j in range(2):
                        mt = mp * 2 + j
                        nc.vector.scalar_tensor_tensor(
                            out=xr2[:, mt, :], in0=f2ps[j][:],
                            scalar=bf2_sb[:, mt : mt + 1], in1=xp_bf[:, mt, :],
                            op0=AluOpType.add, op1=AluOpType.add,
                        )
                x_bf_next = sb.tile([128, KT, R], BF16, tag="xbf", bufs=1, name="x_bfn")
                x_f_next = sb.tile([128, KT, OWN], BF16, tag="xf", bufs=1, name="x_fn")
                ln_T(xr2, ln2s_sb, ln2b_sb, x_f_next, x_bf_next, 256)
                x_bf_cur, x_f_cur = x_bf_next, x_f_next

            # ================= LSTM / pooling tail =================
            send2 = dram.tile([D, OWN], BF16, tag="send", name="send2")
            nc.sync.dma_start(
                out=send2.rearrange("(k p) c -> p k c", p=128),
                in_=x_bf_cur[:, :, 256 : 256 + OWN],
            )
            gath2 = dram.tile([NC * D, OWN], BF16, addr_space="Shared", tag="gath", name="gath2")
            nc.gpsimd.collective_compute(
                "AllGather", AluOpType.bypass, ins=[send2.opt()], outs=[gath2.opt()],
                replica_groups=[list(range(NC))],
            )
            xbT = []
            for bi, (tag, sv) in enumerate((("qT", qD_sv), ("ctxT", q4D_sv))):
                t = sb.tile([128, KT, OWN], BF16, tag=tag, bufs=1, name=f"xb{bi}")
                nc.sync.dma_start(
                    out=t[:, :, :],
                    in_=gath2[ds(sv, D), :].rearrange("(k p) c -> p k c", p=128),
                )
                xbT.append(t)

            # LSTM1 weights (reuse big encoder tags, free by now)
            l1w_sb, l1h_sb, l1b_sb = [], [], []
            for d_, tag in ((0, "kT"), (1, "xbf")):
                wt = sb.tile([128, KT, 1024], BF16, tag=tag, bufs=1, name=f"l1w{d_}")
                nc.sync.dma_start(
                    out=wt[:], in_=l1wih.ap()[d_].rearrange("(k p) m -> p k m", p=128)
                )
                l1w_sb.append(wt)
                ht = sb.tile([128, 2, 1024], BF16, tag="wf1t", bufs=2, name=f"l1h{d_}")
                nc.sync.dma_start(
                    out=ht[:], in_=l1whh.ap()[d_].rearrange("(k p) m -> p k m", p=128)
                )
                l1h_sb.append(ht)
                b1 = sb.tile([128, 8], F32, tag="l1b", bufs=4, name=f"l1bi{d_}")
                nc.sync.dma_start(
                    out=b1[:], in_=l1bih.ap()[d_].rearrange("(k p) -> p k", p=128)
                )
                b2 = sb.tile([128, 8], F32, tag="l1b", bufs=4, name=f"l1bh{d_}")
                nc.sync.dma_start(
                    out=b2[:], in_=l1bhh.ap()[d_].rearrange("(k p) -> p k", p=128)
                )
                nc.vector.tensor_tensor(out=b1[:], in0=b1[:], in1=b2[:], op=AluOpType.add)
                l1b_sb.append(b1)

            GATE_F = {0: AF.Sigmoid, 1: AF.Sigmoid, 2: AF.Tanh, 3: AF.Sigmoid}

            def lstm_step(w_sb, h_sb, b_sb, x_in, nkt, h_prev_bf, c_prev, nn):
                """One LSTM step in T layout; returns (h_bf16, c_f32) [128,2,nn]."""
                gates = []
                for mt in range(8):
                    gp = ps_big.tile([128, OWN], F32, tag="big", name="gp")
                    for i in range(nkt):
                        nc.tensor.matmul(
                            gp[:, :nn], lhsT=w_sb[:, i, ts(mt, 128)],
                            rhs=x_in[:, i, :nn],
                            start=(i == 0),
                            stop=(h_prev_bf is None and i == nkt - 1),
                        )
                    if h_prev_bf is not None:
                        for i in range(2):
                            nc.tensor.matmul(
                                gp[:, :nn], lhsT=h_sb[:, i, ts(mt, 128)],
                                rhs=h_prev_bf[:, i, :nn],
                                start=False, stop=(i == 1),
                            )
                    g = t_exp_bf(f"g{mt}")
                    nc.scalar.activation(
                        g[:, :nn], gp[:, :nn], GATE_F[mt // 2],
                        bias=b_sb[:, mt : mt + 1],
                    )
                    gates.append(g)
                h_b = sb.tile([128, 2, OWN], BF16, tag="exp", bufs=10, name="h_b")
                c_f = sb.tile([128, 2, OWN], F32, tag="wqkvo", bufs=2, name="c_f")
                for i in range(2):
                    tg = sb.tile([128, OWN], F32, tag="wf2t", bufs=2, name="tg")
                    nc.vector.tensor_tensor(
                        out=tg[:, :nn], in0=gates[0 + i][:, :nn],
                        in1=gates[4 + i][:, :nn], op=AluOpType.mult,
                    )  # sig(i)*tanh(g)
                    if c_prev is not None:
                        t2 = sb.tile([128, OWN], F32, tag="wf2t", bufs=2, name="t2")
                        nc.vector.tensor_tensor(
                            out=t2[:, :nn], in0=gates[2 + i][:, :nn],
                            in1=c_prev[:, i, :nn], op=AluOpType.mult,
                        )  # sig(f)*c
                        nc.vector.tensor_tensor(
                            out=c_f[:, i, :nn], in0=tg[:, :nn], in1=t2[:, :nn],
                            op=AluOpType.add,
                        )
                    else:
                        nc.vector.tensor_copy(c_f[:, i, :nn], tg[:, :nn])
                    tc_ = sb.tile([128, OWN], F32, tag="wf2t", bufs=2, name="tc_")
                    nc.scalar.activation(tc_[:, :nn], c_f[:, i, :nn], AF.Tanh)
                    nc.vector.tensor_tensor(
                        out=h_b[:, i, :nn], in0=gates[6 + i][:, :nn],
                        in1=tc_[:, :nn], op=AluOpType.mult,
                    )
                return h_b, c_f

            fh0b, fc0 = lstm_step(l1w_sb[0], l1h_sb[0], l1b_sb[0], xbT[0], KT, None, None, OWN)
            fh1b, _ = lstm_step(l1w_sb[0], l1h_sb[0], l1b_sb[0], xbT[1], KT, fh0b, fc0, OWN)
            bh0b, bc0 = lstm_step(l1w_sb[1], l1h_sb[1], l1b_sb[1], xbT[1], KT, None, None, OWN)
            bh1b, _ = lstm_step(l1w_sb[1], l1h_sb[1], l1b_sb[1], xbT[0], KT, bh0b, bc0, OWN)

            # lstm1 output per t in T layout, as 4 slices [128, OWN] (kt = feat tile)
            def l1rhs(t_, kt):
                fh, bh = (fh0b, bh1b) if t_ == 0 else (fh1b, bh0b)
                return fh[:, kt, :] if kt < 2 else bh[:, kt - 2, :]

            # natural-layout copies via PE transpose: l1N[t_] [128(cols), 4, 512(feat)]
            l1N = []
            for t_, tag in ((0, "xrbf"), (1, "xsq")):
                ln_ = sb.tile([128, 4, OWN], BF16, tag=tag, bufs=1, name=f"l1N{t_}")
                for ft in range(4):
                    for ct in range(4):
                        tp2 = ps_sps.tile([128, 128], BF16, tag="sps", name="tp2")
                        nc.tensor.transpose(tp2[:], l1rhs(t_, ft)[:, ts(ct, 128)], ident[:])
                        nc.scalar.activation(ln_[:, ct, ts(ft, 128)], tp2[:], AF.Copy)
                l1N.append(ln_)

            # attention pooling partials
            paw1_sb = sb.tile([128, 4, 64], BF16, tag="paw1", bufs=1, name="paw1_sb")
            nc.sync.dma_start(
                out=paw1_sb[:], in_=paw1.ap().rearrange("(k p) m -> p k m", p=128)
            )
            paw2_sb = sb.tile([64, 1], BF16, tag="paw2", bufs=1, name="paw2_sb")
            nc.sync.dma_start(out=paw2_sb[:], in_=paw2.ap())
            pab1_sb = sb.tile([64, 1], F32, tag="pab1", bufs=1, name="pab1_sb")
            nc.sync.dma_start(out=pab1_sb[:], in_=pab1.ap())
            pab2_sb = sb.tile([1, 1], F32, tag="pab2", bufs=1, name="pab2_sb")
            nc.sync.dma_start(out=pab2_sb[:], in_=pab2.ap())
            pab2_full = sb.tile([128, 1], F32, tag="pab2f", bufs=1, name="pab2_full")
            nc.gpsimd.partition_broadcast(pab2_full[:], pab2_sb[:])

            st = sb.tile([128, 10], F32, tag="st", bufs=1, name="st")
            nc.vector.memset(st[:], 0.0)
            for t_ in range(2):
                rp = ps_sps.tile([64, OWN], F32, tag="sps", name="rp")
                for kt in range(4):
                    nc.tensor.matmul(
                        rp[:], lhsT=paw1_sb[:, kt, :], rhs=l1rhs(t_, kt),
                        start=(kt == 0), stop=(kt == 3),
                    )
                relu_bf = sb.tile([64, OWN], BF16, tag="relu", bufs=2, name="relu_bf")
                nc.scalar.activation(relu_bf[:], rp[:], AF.Relu, bias=pab1_sb[:, :1])
                wcol = sb.tile([128, 4, 1], BF16, tag="wcol", bufs=2, name="wcol")
                for ct in range(4):
                    ep = ps_sps.tile([128, 1], F32, tag="sps", name="ep")
                    nc.tensor.matmul(
                        ep[:], lhsT=relu_bf[:, ts(ct, 128)], rhs=paw2_sb[:],
                        start=True, stop=True,
                    )
                    etmp = sb.tile([128, 1], F32, tag="etmp", bufs=4, name="etmp")
                    nc.vector.tensor_tensor(
                        out=etmp[:], in0=ep[:], in1=pab2_full[:], op=AluOpType.add
                    )
                    nc.scalar.activation(wcol[:, ct, :], etmp[:], AF.Exp)
                for mt in range(4):
                    pn = ps_sps.tile([128, 1], F32, tag="sps", name="pn")
                    for ct in range(4):
                        nc.tensor.matmul(
                            pn[:], lhsT=l1N[t_][:, ct, ts(mt, 128)], rhs=wcol[:, ct, :],
                            start=(ct == 0), stop=(ct == 3),
                        )
                    nc.scalar.activation(
                        st[:, t_ * 5 + mt : t_ * 5 + mt + 1], pn[:], AF.Copy
                    )
                pd = ps_sps.tile([1, 1], F32, tag="sps", name="pd")
                for ct in range(4):
                    nc.tensor.matmul(
                        pd[:], lhsT=wcol[:, ct, :], rhs=ones_col[:],
                        start=(ct == 0), stop=(ct == 3),
                    )
                nc.scalar.activation(st[0:1, t_ * 5 + 4 : t_ * 5 + 5], pd[:], AF.Copy)

            ar_in = dram.tile([128, 10], F32, tag="arin", name="ar_in")
            nc.sync.dma_start(out=ar_in[:], in_=st[:])
            ar_out = dram.tile([128, 10], F32, addr_space="Shared", tag="arout", name="ar_out")
            nc.gpsimd.collective_compute(
                "AllReduce", AluOpType.add, ins=[ar_in.opt()], outs=[ar_out.opt()],
                replica_groups=[list(range(NC))],
            )
            ared = sb.tile([128, 10], F32, tag="ared", bufs=1, name="ared")
            nc.sync.dma_start(out=ared[:], in_=ar_out[:])

            pooledT = sb.tile([128, 4, 2], BF16, tag="pooledT", bufs=1, name="pooledT")
            for t_ in range(2):
                dn = sb.tile([1, 1], F32, tag="dn", bufs=4, name="dn")
                nc.vector.reciprocal(dn[:], ared[0:1, t_ * 5 + 4 : t_ * 5 + 5])
                dnb = sb.tile([128, 1], F32, tag="dnb", bufs=4, name="dnb")
                nc.gpsimd.partition_broadcast(dnb[:], dn[:])
                for mt in range(4):
                    nc.vector.tensor_tensor(
                        out=pooledT[:, mt, t_ : t_ + 1],
                        in0=ared[:, t_ * 5 + mt : t_ * 5 + mt + 1],
                        in1=dnb[:], op=AluOpType.mult,
                    )

            # LSTM2: T=2 (batch), N=1, input 512
            l2w_sb, l2h_sb, l2b_sb = [], [], []
            for d_, wtag_ in ((0, "xrbf"), (1, "xsq")):
                wt = sb.tile([128, 4, 1024], BF16, tag=wtag_, bufs=1, name=f"l2w{d_}")
                nc.sync.dma_start(
                    out=wt[:], in_=l2wih.ap()[d_].rearrange("(k p) m -> p k m", p=128)
                )
                l2w_sb.append(wt)
                ht = sb.tile([128, 2, 1024], BF16, tag="wf1t", bufs=2, name=f"l2h{d_}")
                nc.sync.dma_start(
                    out=ht[:], in_=l2whh.ap()[d_].rearrange("(k p) m -> p k m", p=128)
                )
                l2h_sb.append(ht)
                b1 = sb.tile([128, 8], F32, tag="l2b", bufs=4, name=f"l2bi{d_}")
                nc.sync.dma_start(
                    out=b1[:], in_=l2bih.ap()[d_].rearrange("(k p) -> p k", p=128)
                )
                b2 = sb.tile([128, 8], F32, tag="l2b", bufs=4, name=f"l2bh{d_}")
                nc.sync.dma_start(
                    out=b2[:], in_=l2bhh.ap()[d_].rearrange("(k p) -> p k", p=128)
                )
                nc.vector.tensor_tensor(out=b1[:], in0=b1[:], in1=b2[:], op=AluOpType.add)
                l2b_sb.append(b1)

            def l2_step(d_, t_src, h_prev_bf, c_prev):
                gates = []
                for mt in range(8):
                    gp2 = ps_sps.tile([128, 1], F32, tag="sps", name="gp2")
                    for kt in range(4):
                        nc.tensor.matmul(
                            gp2[:], lhsT=l2w_sb[d_][:, kt, ts(mt, 128)],
                            rhs=pooledT[:, kt, t_src : t_src + 1],
                            start=(kt == 0), stop=(h_prev_bf is None and kt == 3),
                        )
                    if h_prev_bf is not None:
                        for i in range(2):
                            nc.tensor.matmul(
                                gp2[:], lhsT=l2h_sb[d_][:, i, ts(mt, 128)],
                                rhs=h_prev_bf[:, i, :], start=False, stop=(i == 1),
                            )
                    g = sb.tile([128, 1], F32, tag="g2", bufs=10, name="g2")
                    nc.scalar.activation(
                        g[:], gp2[:], GATE_F[mt // 2], bias=l2b_sb[d_][:, mt : mt + 1]
                    )
                    gates.append(g)
                h_b = sb.tile([128, 2, 1], BF16, tag="hc2b", bufs=6, name="h2b")
                c_f = sb.tile([128, 2, 1], F32, tag="hc2", bufs=6, name="c2f")
                for i in range(2):
                    tg = sb.tile([128, 1], F32, tag="tg2", bufs=4, name="tg2")
                    nc.vector.tensor_tensor(
                        out=tg[:], in0=gates[0 + i][:, :1], in1=gates[4 + i][:, :1],
                        op=AluOpType.mult,
                    )
                    if c_prev is not None:
                        t2 = sb.tile([128, 1], F32, tag="tg2", bufs=4, name="t22")
                        nc.vector.tensor_tensor(
                            out=t2[:], in0=gates[2 + i][:, :1], in1=c_prev[:, i, :],
                            op=AluOpType.mult,
                        )
                        nc.vector.tensor_tensor(
                            out=c_f[:, i, :], in0=tg[:], in1=t2[:], op=AluOpType.add
                        )
                    else:
                        nc.vector.tensor_copy(c_f[:, i, :], tg[:])
                    tc_ = sb.tile([128, 1], F32, tag="tg2", bufs=4, name="tc2")
                    nc.scalar.activation(tc_[:], c_f[:, i, :], AF.Tanh)
                    nc.vector.tensor_tensor(
                        out=h_b[:, i, :], in0=gates[6 + i][:, :1], in1=tc_[:],
                        op=AluOpType.mult,
                    )
                return h_b, c_f

            f2h0b, f2c0 = l2_step(0, 0, None, None)
            f2h1b, _ = l2_step(0, 1, f2h0b, f2c0)
            b2h0b, b2c0 = l2_step(1, 1, None, None)
            b2h1b, _ = l2_step(1, 0, b2h0b, b2c0)

            # flatT [128, 4, 2]: t0 = [f2h0, b2h1], t1 = [f2h1, b2h0]
            flatT = sb.tile([128, 4, 2], BF16, tag="flatT", bufs=1, name="flatT")
            for t_, (fh, bh) in enumerate([(f2h0b, b2h1b), (f2h1b, b2h0b)]):
                for i in range(2):
                    nc.vector.tensor_copy(flatT[:, i, t_ : t_ + 1], fh[:, i, :])
                    nc.vector.tensor_copy(flatT[:, 2 + i, t_ : t_ + 1], bh[:, i, :])

            wtag_sb = sb.tile([128, 4, 1], BF16, tag="wtag", bufs=1, name="wtag_sb")
            nc.sync.dma_start(
                out=wtag_sb[:], in_=wtag.ap().rearrange("(k p) m -> p k m", p=128)
            )
            btag_sb = sb.tile([1, 1], F32, tag="btag", bufs=1, name="btag_sb")
            nc.sync.dma_start(out=btag_sb[:], in_=btag.ap())
            fo = ps_sps.tile([1, 2], F32, tag="sps", name="fo")
            for kt in range(4):
                nc.tensor.matmul(
                    fo[:], lhsT=wtag_sb[:, kt, :], rhs=flatT[:, kt, :],
                    start=(kt == 0), stop=(kt == 3),
                )
            out_sb = sb.tile([1, 2], F32, tag="outsb", bufs=1, name="out_sb")
            nc.scalar.activation(out_sb[:], fo[:], AF.Identity, bias=btag_sb[:1, :1])
            nc.sync.dma_start(out=out.ap(), in_=out_sb[:])

    nc.finalize()
    return nc


def prep_inputs(inputs, L=LMAX):
    """Host-side sharding/layout prep. Returns in_maps for 8 cores."""
    f32 = lambda a: np.ascontiguousarray(np.asarray(a), dtype=np.float32)
    bf = lambda a: np.ascontiguousarray(np.asarray(np.asarray(a), dtype=np.float32).astype(BF))

    sentences = np.asarray(inputs["sentences"], dtype=np.int32)
    shared = {
        "word_emb": bf(inputs["word_emb"]),
        "embs": f32(inputs["emb_ln_s"]).reshape(1, D),
        "embb": f32(inputs["emb_ln_b"]).reshape(1, D),
        "wq": bf(inputs["Wq"][:L]), "wk": bf(inputs["Wk"][:L]),
        "wv": bf(inputs["Wv"][:L]), "wo": bf(inputs["Wo"][:L]),
        "bq": f32(inputs["bq"][:L]), "bk": f32(inputs["bk"][:L]),
        "bo": f32(
            np.asarray(inputs["bo"][:L], dtype=np.float32)
            + np.einsum(
                "ld,ldo->lo",
                np.asarray(inputs["bv"][:L], dtype=np.float32),
                np.asarray(inputs["Wo"][:L], dtype=np.float32),
            )
        ),
        "ln1s": f32(inputs["ln1_s"][:L]), "ln1b": f32(inputs["ln1_b"][:L]),
        "ln2s": f32(inputs["ln2_s"][:L]), "ln2b": f32(inputs["ln2_b"][:L]),
        "wf1": bf(inputs["Wf1"][:L]), "bf1": f32(inputs["bf1"][:L]),
        "wf2": bf(inputs["Wf2"][:L]), "bf2": f32(inputs["bf2"][:L]),
        "l1wih": bf(np.transpose(np.asarray(inputs["l1_Wih"]), (0, 2, 1))),
        "l1whh": bf(np.transpose(np.asarray(inputs["l1_Whh"]), (0, 2, 1))),
        "l1bih": f32(inputs["l1_bih"]), "l1bhh": f32(inputs["l1_bhh"]),
        "paw1": bf(inputs["pa_W1"]), "pab1": f32(inputs["pa_b1"]).reshape(64, 1),
        "paw2": bf(inputs["pa_W2"]), "pab2": f32(inputs["pa_b2"]).reshape(1, 1),
        "l2wih": bf(np.transpose(np.asarray(inputs["l2_Wih"]), (0, 2, 1))),
        "l2whh": bf(np.transpose(np.asarray(inputs["l2_Whh"]), (0, 2, 1))),
        "l2bih": f32(inputs["l2_bih"]), "l2bhh": f32(inputs["l2_bhh"]),
        "wtag": bf(inputs["Wtag"]), "btag": f32(inputs["btag"]).reshape(1, 1),
    }
    pos_emb = f32(inputs["pos_emb"])

    in_maps = []
    for g in range(NC):
        b, q = g // 4, g % 4
        pos0 = 512 * q - 256
        rows = np.arange(R) + pos0
        rows_c = np.clip(rows, 0, S - 1)
        m = dict(shared)
        m["sent_idx"] = np.ascontiguousarray(
            sentences[b][rows_c].reshape(R, 1).astype(np.int32)
        )
        m["pos_sl"] = np.ascontiguousarray(pos_emb[rows_c])
        # masks [2 chunks, 4 slots, 128 keys(p), 256 queries(i)] — 0/1 validity
        mk = np.zeros((2, 4, 128, 256), dtype=np.float32)
        slot_wk = [0, 1, 4, 5]
        i_idx = np.arange(256)
        p_idx = np.arange(128)
        for c in range(2):
            qpos = 512 * q + 256 * c + i_idx  # [256] global query positions
            for s_, wk_ in enumerate(slot_wk):
                kpos = pos0 + 256 * c + wk_ * 128 + p_idx  # [128] global key pos
                valid = (
                    (np.abs(kpos[:, None] - qpos[None, :]) <= W)
                    & (kpos[:, None] >= 0)
                    & (kpos[:, None] < S)
                )
                mk[c, s_][valid] = 1.0
        m["masks"] = np.ascontiguousarray(mk.reshape(2 * 4 * 128, 256).astype(BF))
        # picks [4, KT, 128]: left-halo rank, right-halo rank, batch0 rank, batch1 rank
        d = np.arange(D)
        gl = g - 1 if q > 0 else g
        gr = g + 1 if q < 3 else g
        pk = np.stack(
            [gl * D + d, gr * D + d, q * D + d, (q + 4) * D + d]
        ).reshape(4, KT, 128)
        m["picks"] = np.ascontiguousarray(pk.reshape(4 * KT * 128, 1).astype(np.int32))
        in_maps.append(m)
    return in_maps


_NC_CACHE = {}


def run(inputs, L=LMAX, trace=False):
    if L not in _NC_CACHE:
        _NC_CACHE[L] = build_nc(L)
    nc = _NC_CACHE[L]
    in_maps = prep_inputs(inputs, L)
    res = run_bass_kernel_spmd(nc, in_maps, core_ids=list(range(NC)), trace=trace)
    out = np.asarray(res.results[0]["out"], dtype=np.float32).reshape(B, 1)
    return out, res


def kernel(**inputs) -> np.ndarray:
    out, _ = run(inputs, L=LMAX, trace=False)
    return out

